# revision 45
# baseline (speedup 1.0000x reference)
"""Trainium2 Bass kernel for nn_ArithmeticNps (moe_routing).

Strategy
--------
Data-parallel over batch: each of the 8 cores handles 8192 rows; params are
replicated (tiny). All activations live in transposed layout (features on
partitions, batch on the free dim), so the whole MLP chain runs without
transposes; weights fold through adjacent linear layers on the host:

  - state=0 halves se_W1; encoder L2 folds into the state encoder
    (E' = eo_W2 @ se_W1[cv:]), so each slot's pre-relu state-encoding `z`
    (64-dim) is one matmul away from the wrapped inputs.
  - selector key projections fold through se_W2 (M1', M2'), so rule logits
    and contextual-slot logits come straight from relu(z).
  - expert W1 folds through se_W2 (per-expert 128-dim hidden from the two
    selected 64-dim relu(z) vectors), expert W2 folds through the decoder
    de_W1.

Routing: per 512-row tile, the flat argmax over the 24 (rule, slot) logits
is computed per row via PE transpose + DVE max; the tile's mode (dominant
flat index) picks the expert weights/primary slot through dynamic SBUF
slices (offsets come from tiny constant-table matmuls against the mode
one-hot). Rows disagreeing with the mode are counted; if any tile has a
deviant row, one deferred fully-general (masked, all-expert) pass recomputes
the whole core's output. On the observed data distribution every row agrees,
so the general pass never executes.

Heavy matmuls run as float32r (full-rate fp32 PE mode); walrus requires
f32r operands to be *produced* as f32r, so matmul-feeding DRAM params and
compute outputs are declared float32r end-to-end. The general path sticks
to plain f32 via bitcasts (it never runs; speed irrelevant).
"""

import numpy as np

import concourse.bass as bass
import concourse.mybir as mybir
import concourse.tile as tile
from concourse import bacc
from concourse.bass import ds, ts
from concourse.bass_utils import run_bass_kernel_spmd
from concourse.masks import make_identity

F32 = mybir.dt.float32
F32R = mybir.dt.float32r
I32 = mybir.dt.int32
U32 = mybir.dt.uint32
AF = mybir.ActivationFunctionType
ALU = mybir.AluOpType
AX = mybir.AxisListType

B = 65536
NCORES = 8
RPC = B // NCORES          # rows per core
TILE = 512
NT = RPC // TILE           # tiles per core
CV = 256
NR = 8


def _fold_params(P):
    """Host-side weight folding. All O(param) numpy math."""
    f = np.float32
    g = {k: np.asarray(v, f) for k, v in P.items()}
    se_lo = g["se_W1"][CV:]                  # (256, 64)  state=0 kills the top half

    # wrapped-operand pre-activations from A = [v1; 1; v2]
    LA = np.zeros((3, 128), f)
    LA[0, 0:64] = g["eo_W1"][0]
    LA[1, 0:64] = g["eo_b1"]
    LA[1, 64:128] = g["eo_W1"][1] + g["eo_b1"]
    LA[2, 64:128] = g["eo_W1"][0]

    E = (g["eo_W2"] @ se_lo).astype(f)       # (64, 64)
    cz = (g["eo_b2"] @ se_lo + g["se_b1"]).astype(f)
    LZ = np.zeros((128, 128), f)
    LZ[0:64, 0:64] = E
    LZ[64:128, 64:128] = E
    czz = np.concatenate([cz, cz]).astype(f)[:, None]              # (128, 1)

    ope_enc = (np.maximum(g["er_W1"] + g["er_b1"], 0) @ g["er_W2"] + g["er_b2"])
    LOP = (ope_enc @ se_lo + g["se_b1"]).astype(f)                 # (3, 64)

    read1 = (g["body"] @ g["s1_Wq"] + g["s1_bq"]).astype(f)        # (8, 32)
    M1 = (g["se_W2"] @ g["s1_Wk"] @ read1.T / np.sqrt(32.0)).astype(f)
    c1p = (((g["se_b2"] @ g["s1_Wk"] + g["s1_bk"]) @ read1.T) / np.sqrt(32.0)).astype(f)[:, None]
    q2t = (g["body"] @ g["s2_Wq"] + g["s2_bq"]).astype(f)          # (8, 16)
    M2 = (g["se_W2"] @ g["s2_Wk"] @ q2t.T / np.sqrt(16.0)).astype(f)
    LH = np.concatenate([M1, M2], axis=1).astype(f)                # (64, 16)

    rh_W1, rh_b1 = g["rh_W1"], g["rh_b1"]
    rh_W2, rh_b2 = g["rh_W2"], g["rh_b2"]
    de_W1, de_b1 = g["de_W1"], g["de_b1"]
    W1a = np.zeros((64, NR * 128), f)
    W1b = np.zeros((64, NR * 128), f)
    b1e = np.zeros((128, NR), f)
    W2f = np.zeros((128, NR * 64), f)
    b2e = np.zeros((64, NR), f)
    WE = np.zeros((128, NR * 200), f)
    for e in range(NR):
        W1a[:, e * 128:(e + 1) * 128] = g["se_W2"] @ rh_W1[e][:CV]
        W1b[:, e * 128:(e + 1) * 128] = g["se_W2"] @ rh_W1[e][CV:]
        b1e[:, e] = rh_b1[e] + g["se_b2"] @ rh_W1[e][:CV] + g["se_b2"] @ rh_W1[e][CV:]
        W2f[:, e * 64:(e + 1) * 64] = rh_W2[e] @ de_W1
        b2e[:, e] = rh_b2[e] @ de_W1 + de_b1
        o = e * 200
        WE[0:64, o:o + 128] = W1a[:, e * 128:(e + 1) * 128]
        WE[64:128, o:o + 128] = W1b[:, e * 128:(e + 1) * 128]
        WE[:, o + 128:o + 192] = W2f[:, e * 64:(e + 1) * 64]
        WE[:, o + 192] = b1e[:, e]
        WE[0:64, o + 193] = b2e[:, e]
    LF = g["de_W2"].astype(f)                                      # (64, 1)
    deb2 = np.array([[g["de_b2"][0]]], f)

    # 96-bin flat index: j = 32*slot + rule (rows 8..31 of each block are pads)
    S8 = np.zeros((96, NR), f)
    CE = np.zeros((96, 1), f)
    CP = np.zeros((96, 1), f)
    for t in range(3):
        for n in range(8):
            j = 32 * t + n
            S8[j, n] = 1.0
            CE[j, 0] = WE_OFF + n * 200
            CP[j, 0] = t * 512

    parts = dict(LA=LA, LZ=LZ, czz=czz, LOP=LOP, LH=LH, c1p=c1p,
                 W1a=W1a, W1b=W1b, b1e=b1e, W2f=W2f, b2e=b2e, LF=LF,
                 deb2=deb2, S8=S8, WE=WE, CE=CE, CP=CP)
    W = sum(v.shape[1] for v in parts.values())
    PAR = np.zeros((128, W), f)
    off = 0
    for k in PAR_ORDER:
        v = parts[k]
        PAR[:v.shape[0], off:off + v.shape[1]] = v
        off += v.shape[1]
    return PAR


# name -> (shape, dtype). f32r for anything feeding an f32r matmul.
# All packed into one (128, W) f32r DRAM tensor "PAR" (single DMA).
PARAM_SPECS = dict(
    LA=((3, 128), F32R), LZ=((128, 128), F32R), LOP=((3, 64), F32R),
    LH=((64, 16), F32R), WE=((128, NR * 200), F32R), LF=((64, 1), F32R),
    S8=((96, NR), F32), CE=((96, 1), F32), CP=((96, 1), F32),
    czz=((128, 1), F32), c1p=((8, 1), F32), deb2=((1, 1), F32),
    W1a=((64, NR * 128), F32), W1b=((64, NR * 128), F32),
    b1e=((128, NR), F32), W2f=((128, NR * 64), F32), b2e=((64, NR), F32),
)
PAR_ORDER = list(PARAM_SPECS.keys())
PAR_W = sum(shp[1] for shp, _ in PARAM_SPECS.values())
WE_OFF = sum(PARAM_SPECS[k][0][1] for k in PAR_ORDER[:PAR_ORDER.index("WE")])


def build_nc(force_fallback=False, emit_fallback=True):
    nc = bacc.Bacc("TRN2", target_bir_lowering=False, debug=False)
    A_ap = nc.dram_tensor("A", [3, RPC], F32R, kind="ExternalInput").ap()
    op3_ap = nc.dram_tensor("op3", [3, RPC], F32, kind="ExternalInput").ap()
    p_ap = nc.dram_tensor("PAR", [128, PAR_W], F32R, kind="ExternalInput").ap()
    out_ap = nc.dram_tensor("out", [1, RPC], F32, kind="ExternalOutput").ap()

    with tile.TileContext(nc) as tc:
        with tc.tile_pool(name="const", bufs=1) as cpool, \
             tc.tile_pool(name="work", bufs=2) as wpool, \
             tc.tile_pool(name="gwork", bufs=1) as gpool, \
             tc.tile_pool(name="psum", bufs=2, space="PSUM") as ppool, \
             tc.tile_pool(name="psum_sm", bufs=3, space="PSUM") as pps, \
             tc.tile_pool(name="psum_t", bufs=1, space="PSUM") as ppt:
            _emit_body(tc, nc, A_ap, op3_ap, p_ap, out_ap, force_fallback,
                       emit_fallback, cpool, wpool, gpool, ppool, pps, ppt)
    nc.compile()
    return nc


def _emit_body(tc, nc, A_ap, op3_ap, p_ap, out_ap, force_fallback,
               emit_fallback, cpool, wpool, gpool, ppool, pps, ppt):
    # ---- constants: one packed param DMA, per-param slice views ----
    par = cpool.tile([128, PAR_W], F32R, tag="par")
    nc.sync.dma_start(par[:], p_ap[:])
    pc = {}
    off = 0
    for k in PAR_ORDER:
        (rows, cols), dt = PARAM_SPECS[k]
        v = par[0:rows, off:off + cols]
        pc[k] = v if dt == F32R else v.bitcast(F32)
        off += cols
    ones = cpool.tile([128, 1], F32, tag="ones")       # column of ones
    nc.gpsimd.memset(ones[:], 1.0)
    onesr = cpool.tile([1, 128], F32, tag="onesr")     # f32 ones row
    nc.gpsimd.memset(onesr[:], 1.0)
    onesrr = cpool.tile([1, 64], F32R, tag="onesrr")   # f32r ones row (m1 bcast)
    nc.vector.tensor_copy(onesrr[:], onesr[:, 0:64])
    id128 = cpool.tile([128, 128], F32, tag="id128")
    make_identity(nc, id128[:])
    LG = cpool.tile([96, 512], F32, tag="LG")
    nc.gpsimd.memset(LG[:], -1e30)
    iota3 = cpool.tile([3, TILE], F32, tag="iota3")    # value = partition idx
    nc.gpsimd.iota(iota3[:], pattern=[[0, TILE]], base=0, channel_multiplier=1,
                   allow_small_or_imprecise_dtypes=True)
    out_sb = cpool.tile([1, RPC], F32, tag="out_sb")
    cms = cpool.tile([1, NT], F32, tag="cms")          # per-tile mode count
    tc.strict_bb_all_engine_barrier()

    # ---------------- happy path over tiles (2-stage software pipeline) ----------------
    def phase_a(i):
        """Encode, route, stage weights for tile i. Returns state for phase_b."""
        ch = ts(i, TILE)
        at = wpool.tile([3, TILE], F32R, tag="at")
        nc.sync.dma_start(at[:], A_ap[:, ch])
        opt = wpool.tile([3, TILE], F32, tag="opt")
        nc.sync.dma_start(opt[:], op3_ap[:, ch])

        # encoders -> per-slot pre-state z (64) stacked on free dim
        psA = ppool.tile([128, TILE], F32, tag="big")
        nc.tensor.matmul(psA[:], pc["LA"], at[:], start=True, stop=True)
        ra = wpool.tile([128, TILE], F32R, tag="ra")
        nc.scalar.activation(ra[:], psA[:], AF.Relu)
        psZ = ppool.tile([128, TILE], F32, tag="big")
        nc.tensor.matmul(psZ[:], pc["LZ"], ra[:], start=True, stop=True)
        oh3 = wpool.tile([3, TILE], F32R, tag="oh3")
        nc.vector.tensor_tensor(oh3[:], iota3[:], opt[:], ALU.is_equal)
        psOP = ppool.tile([64, TILE], F32, tag="med")
        nc.tensor.matmul(psOP[:], pc["LOP"], oh3[:], start=True, stop=True)

        zs = wpool.tile([64, 3 * TILE], F32R, tag="zs")
        nc.scalar.activation(zs[:, 0:TILE], psZ[0:64, :], AF.Relu, bias=pc["czz"][0:64, :])
        nc.scalar.activation(zs[:, TILE:2 * TILE], psZ[64:128, :], AF.Relu, bias=pc["czz"][64:128, :])
        nc.scalar.activation(zs[:, 2 * TILE:3 * TILE], psOP[:], AF.Relu)

        # heads: logits into padded 96-row LG (pads stay -1e30); d via z-delta
        psL0 = pps.tile([8, TILE], F32, tag="sm")
        psL1 = pps.tile([8, TILE], F32, tag="sm")
        psHc = pps.tile([8, TILE], F32, tag="sm")
        nc.tensor.matmul(psL0[:], pc["LH"][:, 0:8], zs[:, 0:TILE], start=True, stop=True)
        nc.tensor.matmul(psL1[:], pc["LH"][:, 0:8], zs[:, TILE:2 * TILE], start=True, stop=True)
        nc.tensor.matmul(psHc[:], pc["LH"][:, 0:8], zs[:, 2 * TILE:3 * TILE], start=True, stop=True)
        nc.scalar.activation(LG[0:8, :], psL0[:], AF.Identity, bias=pc["c1p"])
        nc.scalar.activation(LG[32:40, :], psL1[:], AF.Identity, bias=pc["c1p"])
        nc.scalar.activation(LG[64:72, :], psHc[:], AF.Identity, bias=pc["c1p"])
        vd = wpool.tile([64, TILE], F32R, tag="vd")
        nc.vector.tensor_tensor(vd[:], zs[:, TILE:2 * TILE].bitcast(F32),
                                zs[:, 0:TILE].bitcast(F32), ALU.subtract)
        psSD = pps.tile([8, TILE], F32, tag="sm")
        nc.tensor.matmul(psSD[:], pc["LH"][:, 8:16], vd[:], start=True, stop=True)
        sd = wpool.tile([8, TILE], F32R, tag="sd")
        nc.vector.tensor_copy(sd[:], psSD[:])

        # per-row flat argmax via transpose + free-dim max (96-bin padded)
        psT = ppt.tile([128, 384], F32, tag="smT")
        for j in range(4):
            nc.tensor.transpose(psT[:, ts(j, 96)], LG[:, ts(j, 128)], id128[0:96, 0:96])
        mx = wpool.tile([128, 32], F32, tag="mx")
        ohn = wpool.tile([128, 384], F32, tag="ohn")
        for j in range(4):
            nc.vector.max(mx[:, ts(j, 8)], psT[:, ts(j, 96)])
            nc.vector.tensor_scalar(ohn[:, ts(j, 96)], psT[:, ts(j, 96)],
                                    mx[:, 8 * j:8 * j + 1], None, ALU.is_equal)
        ohs = wpool.tile([128, 96], F32, tag="ohs")
        nc.vector.tensor_tensor(ohs[:], ohn[:, ts(0, 96)], ohn[:, ts(1, 96)], ALU.add)
        ohs2 = wpool.tile([128, 96], F32, tag="ohs2")
        nc.vector.tensor_tensor(ohs2[:], ohn[:, ts(2, 96)], ohn[:, ts(3, 96)], ALU.add)
        nc.vector.tensor_tensor(ohs[:], ohs[:], ohs2[:], ALU.add)
        psC = pps.tile([96, 1], F32, tag="sm")
        nc.tensor.matmul(psC[:], ohs[:], ones[:], start=True, stop=True)
        cnt = wpool.tile([96, 1], F32, tag="cnt")
        nc.scalar.activation(cnt[:], psC[:], AF.Identity)
        psCT = pps.tile([1, 96], F32, tag="sm")
        nc.tensor.transpose(psCT[:], cnt[:], id128[0:96, 0:96])
        cntr = wpool.tile([1, 96], F32, tag="cntr")
        nc.scalar.activation(cntr[:], psCT[:], AF.Identity)
        cmx = wpool.tile([1, 8], F32, tag="cmx")
        nc.vector.max(cmx[:], cntr[:])
        nc.vector.tensor_copy(cms[:, i:i + 1], cmx[:, 0:1])

        # mode one-hot over the 96 flat bins -> expert sel + offsets
        psCMB = pps.tile([96, 1], F32, tag="sm")
        nc.tensor.matmul(psCMB[:], onesr[:, 0:96], cmx[:, 0:1], start=True, stop=True)
        oh24 = wpool.tile([96, 1], F32, tag="oh24")
        nc.vector.tensor_tensor(oh24[:], cnt[:], psCMB[:], ALU.is_equal)
        psS8 = pps.tile([8, 1], F32, tag="sm")
        nc.tensor.matmul(psS8[:], pc["S8"], oh24[:], start=True, stop=True)
        sel8 = wpool.tile([8, 1], F32R, tag="sel8")
        nc.scalar.activation(sel8[:], psS8[:], AF.Identity)
        psE2 = pps.tile([1, 1], F32, tag="sm")
        nc.tensor.matmul(psE2[:], pc["CE"], oh24[:], start=True, stop=True)
        psP5 = pps.tile([1, 1], F32, tag="sm")
        nc.tensor.matmul(psP5[:], pc["CP"], oh24[:], start=True, stop=True)
        eoff_i = wpool.tile([1, 1], I32, tag="eoff_i")
        nc.vector.tensor_scalar(eoff_i[:], psE2[:], float(WE_OFF + 1400), None, ALU.min)
        poff_i = wpool.tile([1, 1], I32, tag="poff_i")
        nc.vector.tensor_scalar(poff_i[:], psP5[:], 1024.0, None, ALU.min)
        rv_e = nc.values_load(eoff_i[0:1, 0:1], engines=(mybir.EngineType.DVE,),
                              min_val=WE_OFF, max_val=WE_OFF + 1400,
                              skip_runtime_bounds_check=True)
        rv_p = nc.values_load(poff_i[0:1, 0:1], engines=(mybir.EngineType.DVE,),
                              min_val=0, max_val=1024, skip_runtime_bounds_check=True)
        stg = wpool.tile([128, 200], F32R, tag="stg")
        nc.vector.tensor_copy(stg[:], par[:, ds(rv_e, 200)])
        vblk = wpool.tile([128, TILE], F32R, tag="vblk")
        nc.vector.tensor_copy(vblk[0:64, :], zs[:, ds(rv_p, TILE)])
        return dict(i=i, zs=zs, vd=vd, sd=sd, sel8=sel8, stg=stg, vblk=vblk)

    def phase_b1(st):
        """Mask chain: contextual-slot choice -> vblk rows 64:128 for tile st['i']."""
        zs, vd, sd, sel8, vblk = (st[k] for k in ("zs", "vd", "sd", "sel8", "vblk"))
        psDS = pps.tile([1, TILE], F32, tag="sm")
        nc.tensor.matmul(psDS[:], sel8[:], sd[:], start=True, stop=True)
        m1 = wpool.tile([1, TILE], F32R, tag="m1")
        nc.vector.tensor_scalar(m1[:], psDS[:], 0.0, None, ALU.is_gt)
        psM1B = ppool.tile([64, TILE], F32, tag="med")
        nc.tensor.matmul(psM1B[:], onesrr[:], m1[:], start=True, stop=True)
        vm = wpool.tile([64, TILE], F32, tag="vm")
        nc.vector.tensor_tensor(vm[:], vd[:].bitcast(F32), psM1B[:], ALU.mult)
        nc.vector.tensor_tensor(vblk[64:128, :], vm[:], zs[:, 0:TILE].bitcast(F32), ALU.add)
        return st

    def phase_b2(st):
        """Expert MLP + folded decoder for tile st['i']."""
        i, stg, vblk = st["i"], st["stg"], st["vblk"]
        ch = ts(i, TILE)
        psE = ppool.tile([128, TILE], F32, tag="big")
        nc.tensor.matmul(psE[:], stg[:, 0:128], vblk[:], start=True, stop=True)
        he = wpool.tile([128, TILE], F32R, tag="he")
        nc.scalar.activation(he[:], psE[:], AF.Relu, bias=stg[:, 192:193].bitcast(F32))
        psO = ppool.tile([64, TILE], F32, tag="med")
        nc.tensor.matmul(psO[:], stg[:, 128:192], he[:], start=True, stop=True)
        ho = wpool.tile([64, TILE], F32R, tag="ho")
        nc.scalar.activation(ho[:], psO[:], AF.Relu, bias=stg[0:64, 193:194].bitcast(F32))
        psF = pps.tile([1, TILE], F32, tag="sm")
        nc.tensor.matmul(psF[:], pc["LF"], ho[:], start=True, stop=True)
        nc.scalar.activation(out_sb[:, ch], psF[:], AF.Identity, bias=pc["deb2"][0:1, 0:1])

    prev = None
    for i in range(NT):
        b1 = phase_b1(prev) if prev is not None else None
        st = phase_a(i)
        if b1 is not None:
            phase_b2(b1)
        prev = st
    phase_b2(phase_b1(prev))

    # ---------------- deviance check + deferred general path ----------------
    cmn = wpool.tile([1, 1], F32, tag="cmn")
    nc.vector.tensor_reduce(cmn[:], cms[:], AX.X, ALU.min)
    cmn_i = wpool.tile([1, 1], I32, tag="cmn_i")
    nc.vector.tensor_copy(cmn_i[:], cmn[:])
    rv_min = nc.values_load(cmn_i[0:1, 0:1], min_val=0, max_val=TILE,
                            skip_runtime_bounds_check=True)

    if emit_fallback:
        thresh = TILE + 1 if force_fallback else TILE
        with tc.If(rv_min < thresh):
            _emit_general(tc, nc, A_ap, op3_ap, pc, out_sb,
                          cpool, gpool, ppool, pps, ones, onesr, id128, iota3)

    nc.sync.dma_start(out_ap[:], out_sb[:])


def _emit_general(tc, nc, A_ap, op3_ap, pc, out_sb, cpool, wpool, ppool, pps,
                  ones, onesr, id128, iota3):
    """Fully general masked all-expert recompute of the whole core (plain f32).

    Runs only if some tile had a row disagreeing with its mode (never on the
    observed data). Overwrites out_sb entirely.
    """
    iota8 = cpool.tile([8, TILE], F32, tag="iota8")
    nc.gpsimd.iota(iota8[:], pattern=[[0, TILE]], base=0, channel_multiplier=1,
                   allow_small_or_imprecise_dtypes=True)
    for i in range(NT):
        ch = ts(i, TILE)
        at = wpool.tile([3, TILE], F32, tag="g_at")
        nc.sync.dma_start(at[:], A_ap[:, ch].bitcast(F32))
        opt = wpool.tile([3, TILE], F32, tag="g_opt")
        nc.sync.dma_start(opt[:], op3_ap[:, ch])

        psA = ppool.tile([128, TILE], F32, tag="big")
        nc.tensor.matmul(psA[:], pc["LA"].bitcast(F32), at[:], start=True, stop=True)
        ra = wpool.tile([128, TILE], F32, tag="g_ra")
        nc.scalar.activation(ra[:], psA[:], AF.Relu)
        psZ = ppool.tile([128, TILE], F32, tag="big")
        nc.tensor.matmul(psZ[:], pc["LZ"].bitcast(F32), ra[:], start=True, stop=True)
        oh3 = wpool.tile([3, TILE], F32, tag="g_oh3")
        nc.vector.tensor_tensor(oh3[:], iota3[:], opt[:], ALU.is_equal)
        psOP = ppool.tile([64, TILE], F32, tag="med")
        nc.tensor.matmul(psOP[:], pc["LOP"].bitcast(F32), oh3[:], start=True, stop=True)
        zs = wpool.tile([64, 3 * TILE], F32, tag="g_zs")
        nc.scalar.activation(zs[:, 0:TILE], psZ[0:64, :], AF.Relu, bias=pc["czz"][0:64, :])
        nc.scalar.activation(zs[:, TILE:2 * TILE], psZ[64:128, :], AF.Relu, bias=pc["czz"][64:128, :])
        nc.scalar.activation(zs[:, 2 * TILE:3 * TILE], psOP[:], AF.Relu)

        LHf = pc["LH"][:].bitcast(F32)
        psHa = ppool.tile([64, TILE], F32, tag="big")
        psHb = ppool.tile([64, TILE], F32, tag="med")
        psHc = pps.tile([8, TILE], F32, tag="sm")
        nc.tensor.matmul(psHa[0:8, :], LHf[:, 0:8], zs[:, 0:TILE], start=True, stop=True)
        nc.tensor.matmul(psHa[32:40, :], LHf[:, 8:16], zs[:, 0:TILE], start=True, stop=True)
        nc.tensor.matmul(psHb[0:8, :], LHf[:, 0:8], zs[:, TILE:2 * TILE], start=True, stop=True)
        nc.tensor.matmul(psHb[32:40, :], LHf[:, 8:16], zs[:, TILE:2 * TILE], start=True, stop=True)
        nc.tensor.matmul(psHc[:], LHf[:, 0:8], zs[:, 2 * TILE:3 * TILE], start=True, stop=True)
        l0 = wpool.tile([8, TILE], F32, tag="g_l0")
        l1 = wpool.tile([8, TILE], F32, tag="g_l1")
        l2 = wpool.tile([8, TILE], F32, tag="g_l2")
        nc.scalar.activation(l0[:], psHa[0:8, :], AF.Identity, bias=pc["c1p"])
        nc.scalar.activation(l1[:], psHb[0:8, :], AF.Identity, bias=pc["c1p"])
        nc.scalar.activation(l2[:], psHc[:], AF.Identity, bias=pc["c1p"])
        a0s = wpool.tile([8, TILE], F32, tag="g_a0s")
        nc.scalar.activation(a0s[:], psHa[32:40, :], AF.Copy)
        sd = wpool.tile([8, TILE], F32, tag="g_sd")
        nc.vector.tensor_tensor(sd[:], psHb[32:40, :], a0s[:], ALU.subtract)

        # per-row argmax -> slot/rule rows in transposed layout
        psT = pps.tile([128, 96], F32, tag="sm")
        for j in range(4):
            for t, lt in enumerate((l0, l1, l2)):
                nc.tensor.transpose(psT[:, 24 * j + 8 * t:24 * j + 8 * (t + 1)],
                                    lt[:, ts(j, 128)], id128[0:8, 0:8])
        vT = wpool.tile([128, 96], F32, tag="g_vT")
        nc.scalar.activation(vT[:], psT[:], AF.Copy)
        tn = wpool.tile([128, 2], F32, tag="g_tn")      # per-chunk [t, n] cols
        tr_t = wpool.tile([1, TILE], F32, tag="g_trt")  # slot row
        tr_n = wpool.tile([1, TILE], F32, tag="g_trn")  # rule row
        for j in range(4):
            mxj = wpool.tile([128, 8], F32, tag="g_mxj")
            mij = wpool.tile([128, 8], U32, tag="g_mij")
            nc.vector.max_with_indices(mxj[:], mij[:], vT[:, 24 * j:24 * (j + 1)])
            jf = wpool.tile([128, 1], F32, tag="g_jf")
            nc.vector.tensor_copy(jf[:], mij[:, 0:1])
            # t = 1{j>=8} + 1{j>=16};  n = j - 8t
            t1 = wpool.tile([128, 1], F32, tag="g_t1")
            nc.vector.tensor_scalar(t1[:], jf[:], 7.5, None, ALU.is_gt)
            t2 = wpool.tile([128, 1], F32, tag="g_t2")
            nc.vector.tensor_scalar(t2[:], jf[:], 15.5, None, ALU.is_gt)
            nc.vector.tensor_tensor(tn[:, 0:1], t1[:], t2[:], ALU.add)
            t8 = wpool.tile([128, 1], F32, tag="g_t8")
            nc.vector.tensor_scalar(t8[:], tn[:, 0:1], -8.0, None, ALU.mult)
            nc.vector.tensor_tensor(tn[:, 1:2], jf[:], t8[:], ALU.add)
            psTT = pps.tile([1, 128], F32, tag="sm")
            nc.tensor.transpose(psTT[:], tn[:, 0:1], id128[:])
            nc.scalar.activation(tr_t[:, ts(j, 128)], psTT[:], AF.Copy)
            psTN = pps.tile([1, 128], F32, tag="sm")
            nc.tensor.transpose(psTN[:], tn[:, 1:2], id128[:])
            nc.scalar.activation(tr_n[:, ts(j, 128)], psTN[:], AF.Copy)

        # rule one-hot (transposed) + slot row broadcast to 64 partitions
        psNB = pps.tile([8, TILE], F32, tag="sm")
        nc.tensor.matmul(psNB[:], onesr[:, 0:8], tr_n[:], start=True, stop=True)
        ohT8 = wpool.tile([8, TILE], F32, tag="g_ohT8")
        nc.vector.tensor_tensor(ohT8[:], iota8[:], psNB[:], ALU.is_equal)
        psTB = ppool.tile([64, TILE], F32, tag="med")
        nc.tensor.matmul(psTB[:], onesr[:, 0:64], tr_t[:], start=True, stop=True)
        tslot = wpool.tile([64, TILE], F32, tag="g_tslot")
        nc.scalar.activation(tslot[:], psTB[:], AF.Copy)

        # zp = sum_t zs_t * 1{slot==t}
        zp = wpool.tile([64, TILE], F32, tag="g_zp")
        acc = wpool.tile([64, TILE], F32, tag="g_acc")
        ohm = wpool.tile([64, TILE], F32, tag="g_ohm")
        for t in range(3):
            nc.vector.tensor_scalar(ohm[:], tslot[:], float(t), None, ALU.is_equal)
            dst = zp if t == 0 else acc
            nc.vector.tensor_tensor(dst[:], zs[:, t * TILE:(t + 1) * TILE], ohm[:], ALU.mult)
            if t > 0:
                nc.vector.tensor_tensor(zp[:], zp[:], acc[:], ALU.add)
        sdm = wpool.tile([8, TILE], F32, tag="g_sdm")
        nc.vector.tensor_tensor(sdm[:], sd[:], ohT8[:], ALU.mult)
        psDR = pps.tile([1, TILE], F32, tag="sm")
        nc.tensor.matmul(psDR[:], ones[0:8, :], sdm[:], start=True, stop=True)
        m1 = wpool.tile([1, TILE], F32, tag="g_m1")
        nc.vector.tensor_scalar(m1[:], psDR[:], 0.0, None, ALU.is_gt)
        psM1B = ppool.tile([64, TILE], F32, tag="med")
        nc.tensor.matmul(psM1B[:], onesr[:, 0:64], m1[:], start=True, stop=True)
        m1bi = wpool.tile([64, TILE], I32, tag="g_m1bi")
        nc.vector.tensor_copy(m1bi[:], psM1B[:])
        vc = wpool.tile([64, TILE], F32, tag="g_vc")
        nc.vector.select(vc[:], m1bi[:], zs[:, TILE:2 * TILE], zs[:, 0:TILE])

        # all experts, masked combine
        oacc = wpool.tile([1, TILE], F32, tag="g_oacc")
        me = wpool.tile([1, TILE], F32, tag="g_me")
        fm = wpool.tile([1, TILE], F32, tag="g_fm")
        for e in range(NR):
            psE = ppool.tile([128, TILE], F32, tag="big")
            nc.tensor.matmul(psE[:], pc["W1a"][:, ts(e, 128)], zp[:], start=True, stop=False)
            nc.tensor.matmul(psE[:], pc["W1b"][:, ts(e, 128)], vc[:], start=False, stop=True)
            he = wpool.tile([128, TILE], F32, tag="g_he")
            nc.scalar.activation(he[:], psE[:], AF.Relu, bias=pc["b1e"][:, e:e + 1])
            psO = ppool.tile([64, TILE], F32, tag="med")
            nc.tensor.matmul(psO[:], pc["W2f"][:, ts(e, 64)], he[:], start=True, stop=True)
            ho = wpool.tile([64, TILE], F32, tag="g_ho")
            nc.scalar.activation(ho[:], psO[:], AF.Relu, bias=pc["b2e"][:, e:e + 1])
            psF = pps.tile([1, TILE], F32, tag="sm")
            nc.tensor.matmul(psF[:], pc["LF"].bitcast(F32), ho[:], start=True, stop=True)
            fe = wpool.tile([1, TILE], F32, tag="g_fe")
            nc.scalar.activation(fe[:], psF[:], AF.Identity, bias=pc["deb2"][0:1, 0:1])
            nc.vector.tensor_scalar(me[:], tr_n[:], float(e), None, ALU.is_equal)
            nc.vector.tensor_tensor(fm[:], fe[:], me[:], ALU.mult)
            if e == 0:
                nc.vector.tensor_copy(oacc[:], fm[:])
            else:
                nc.vector.tensor_tensor(oacc[:], oacc[:], fm[:], ALU.add)
        nc.vector.tensor_copy(out_sb[:, ch], oacc[:])


def _shard_inputs(operand1, operand2, operator, PAR):
    o1 = np.asarray(operand1, np.float32)
    o2 = np.asarray(operand2, np.float32)
    opf = np.asarray(operator).astype(np.float32)
    in_maps = []
    for c in range(NCORES):
        sl = slice(c * RPC, (c + 1) * RPC)
        A = np.stack([o1[sl], np.ones(RPC, np.float32), o2[sl]])
        op3 = np.repeat(opf[sl][None, :], 3, axis=0)
        in_maps.append({"A": A, "op3": op3, "PAR": PAR})
    return in_maps


_NC_CACHE = {}


def kernel(operand1, operand2, operator, params):
    P = {k: np.asarray(v) for k, v in params.items()}
    PAR = _fold_params(P)
    in_maps = _shard_inputs(operand1, operand2, operator, PAR)
    if "nc" not in _NC_CACHE:
        _NC_CACHE["nc"] = build_nc()
    res = run_bass_kernel_spmd(_NC_CACHE["nc"], in_maps, core_ids=list(range(NCORES)))
    out = np.concatenate([res.results[c]["out"].reshape(-1) for c in range(NCORES)])
    return out.astype(np.float32)


if __name__ == "__main__":
    d = np.load("/tmp/inputs.npz")
    params = {k: d[k] for k in d.files if k not in ("operand1", "operand2", "operator")}
    out = kernel(d["operand1"], d["operand2"], d["operator"], params)
    exp = np.load("/tmp/expected.npy")
    rel = np.linalg.norm(out - exp) / np.linalg.norm(exp)
    print("Relative error:", rel)


# revision 47
# speedup vs baseline: 1.0282x; 1.0282x over previous
"""Trainium2 Bass kernel for nn_ArithmeticNps (moe_routing).

Strategy
--------
Data-parallel over batch: each of the 8 cores handles 8192 rows; params are
replicated (tiny). All activations live in transposed layout (features on
partitions, batch on the free dim), so the whole MLP chain runs without
transposes; weights fold through adjacent linear layers on the host:

  - state=0 halves se_W1; encoder L2 folds into the state encoder
    (E' = eo_W2 @ se_W1[cv:]), so each slot's pre-relu state-encoding `z`
    (64-dim) is one matmul away from the wrapped inputs.
  - selector key projections fold through se_W2 (M1', M2'), so rule logits
    and contextual-slot logits come straight from relu(z).
  - expert W1 folds through se_W2 (per-expert 128-dim hidden from the two
    selected 64-dim relu(z) vectors), expert W2 folds through the decoder
    de_W1.

Routing: per 512-row tile, the flat argmax over the 24 (rule, slot) logits
is computed per row via PE transpose + DVE max; the tile's mode (dominant
flat index) picks the expert weights/primary slot through dynamic SBUF
slices (offsets come from tiny constant-table matmuls against the mode
one-hot). Rows disagreeing with the mode are counted; if any tile has a
deviant row, one deferred fully-general (masked, all-expert) pass recomputes
the whole core's output. On the observed data distribution every row agrees,
so the general pass never executes.

Heavy matmuls run as float32r (full-rate fp32 PE mode); walrus requires
f32r operands to be *produced* as f32r, so matmul-feeding DRAM params and
compute outputs are declared float32r end-to-end. The general path sticks
to plain f32 via bitcasts (it never runs; speed irrelevant).
"""

import numpy as np

import concourse.bass as bass
import concourse.mybir as mybir
import concourse.tile as tile
from concourse import bacc
from concourse.bass import ds, ts
from concourse.bass_utils import run_bass_kernel_spmd
from concourse.masks import make_identity

F32 = mybir.dt.float32
F32R = mybir.dt.float32r
I32 = mybir.dt.int32
U32 = mybir.dt.uint32
AF = mybir.ActivationFunctionType
ALU = mybir.AluOpType
AX = mybir.AxisListType

B = 65536
NCORES = 8
RPC = B // NCORES          # rows per core
TILE = 512
NT = RPC // TILE           # tiles per core
CV = 256
NR = 8


def _fold_params(P):
    """Host-side weight folding. All O(param) numpy math."""
    f = np.float32
    g = {k: np.asarray(v, f) for k, v in P.items()}
    se_lo = g["se_W1"][CV:]                  # (256, 64)  state=0 kills the top half

    # wrapped-operand pre-activations from A = [v1; 1; v2]
    LA = np.zeros((3, 128), f)
    LA[0, 0:64] = g["eo_W1"][0]
    LA[1, 0:64] = g["eo_b1"]
    LA[1, 64:128] = g["eo_W1"][1] + g["eo_b1"]
    LA[2, 64:128] = g["eo_W1"][0]

    E = (g["eo_W2"] @ se_lo).astype(f)       # (64, 64)
    cz = (g["eo_b2"] @ se_lo + g["se_b1"]).astype(f)
    LZ = np.zeros((128, 128), f)
    LZ[0:64, 0:64] = E
    LZ[64:128, 64:128] = E
    czz = np.concatenate([cz, cz]).astype(f)[:, None]              # (128, 1)

    ope_enc = (np.maximum(g["er_W1"] + g["er_b1"], 0) @ g["er_W2"] + g["er_b2"])
    LOP = (ope_enc @ se_lo + g["se_b1"]).astype(f)                 # (3, 64)

    read1 = (g["body"] @ g["s1_Wq"] + g["s1_bq"]).astype(f)        # (8, 32)
    M1 = (g["se_W2"] @ g["s1_Wk"] @ read1.T / np.sqrt(32.0)).astype(f)
    c1p = (((g["se_b2"] @ g["s1_Wk"] + g["s1_bk"]) @ read1.T) / np.sqrt(32.0)).astype(f)[:, None]
    q2t = (g["body"] @ g["s2_Wq"] + g["s2_bq"]).astype(f)          # (8, 16)
    M2 = (g["se_W2"] @ g["s2_Wk"] @ q2t.T / np.sqrt(16.0)).astype(f)
    LH = np.concatenate([M1, M2], axis=1).astype(f)                # (64, 16)

    rh_W1, rh_b1 = g["rh_W1"], g["rh_b1"]
    rh_W2, rh_b2 = g["rh_W2"], g["rh_b2"]
    de_W1, de_b1 = g["de_W1"], g["de_b1"]
    W1a = np.zeros((64, NR * 128), f)
    W1b = np.zeros((64, NR * 128), f)
    b1e = np.zeros((128, NR), f)
    W2f = np.zeros((128, NR * 64), f)
    b2e = np.zeros((64, NR), f)
    WE = np.zeros((128, NR * 200), f)
    for e in range(NR):
        W1a[:, e * 128:(e + 1) * 128] = g["se_W2"] @ rh_W1[e][:CV]
        W1b[:, e * 128:(e + 1) * 128] = g["se_W2"] @ rh_W1[e][CV:]
        b1e[:, e] = rh_b1[e] + g["se_b2"] @ rh_W1[e][:CV] + g["se_b2"] @ rh_W1[e][CV:]
        W2f[:, e * 64:(e + 1) * 64] = rh_W2[e] @ de_W1
        b2e[:, e] = rh_b2[e] @ de_W1 + de_b1
        o = e * 200
        WE[0:64, o:o + 128] = W1a[:, e * 128:(e + 1) * 128]
        WE[64:128, o:o + 128] = W1b[:, e * 128:(e + 1) * 128]
        WE[:, o + 128:o + 192] = W2f[:, e * 64:(e + 1) * 64]
        WE[:, o + 192] = b1e[:, e]
        WE[0:64, o + 193] = b2e[:, e]
    LF = g["de_W2"].astype(f)                                      # (64, 1)
    deb2 = np.array([[g["de_b2"][0]]], f)

    # 96-bin flat index: j = 32*slot + rule (rows 8..31 of each block are pads)
    S8 = np.zeros((96, NR), f)
    CE = np.zeros((96, 1), f)
    CP = np.zeros((96, 1), f)
    for t in range(3):
        for n in range(8):
            j = 32 * t + n
            S8[j, n] = 1.0
            CE[j, 0] = WE_OFF + n * 200
            CP[j, 0] = t * 512

    parts = dict(LA=LA, LZ=LZ, czz=czz, LOP=LOP, LH=LH, c1p=c1p,
                 W1a=W1a, W1b=W1b, b1e=b1e, W2f=W2f, b2e=b2e, LF=LF,
                 deb2=deb2, S8=S8, WE=WE, CE=CE, CP=CP)
    W = sum(v.shape[1] for v in parts.values())
    PAR = np.zeros((128, W), f)
    off = 0
    for k in PAR_ORDER:
        v = parts[k]
        PAR[:v.shape[0], off:off + v.shape[1]] = v
        off += v.shape[1]
    return PAR


# name -> (shape, dtype). f32r for anything feeding an f32r matmul.
# All packed into one (128, W) f32r DRAM tensor "PAR" (single DMA).
PARAM_SPECS = dict(
    LA=((3, 128), F32R), LZ=((128, 128), F32R), LOP=((3, 64), F32R),
    LH=((64, 16), F32R), WE=((128, NR * 200), F32R), LF=((64, 1), F32R),
    S8=((96, NR), F32), CE=((96, 1), F32), CP=((96, 1), F32),
    czz=((128, 1), F32), c1p=((8, 1), F32), deb2=((1, 1), F32),
    W1a=((64, NR * 128), F32), W1b=((64, NR * 128), F32),
    b1e=((128, NR), F32), W2f=((128, NR * 64), F32), b2e=((64, NR), F32),
)
PAR_ORDER = list(PARAM_SPECS.keys())
PAR_W = sum(shp[1] for shp, _ in PARAM_SPECS.values())
WE_OFF = sum(PARAM_SPECS[k][0][1] for k in PAR_ORDER[:PAR_ORDER.index("WE")])


def build_nc(force_fallback=False, emit_fallback=True):
    nc = bacc.Bacc("TRN2", target_bir_lowering=False, debug=False)
    A_ap = nc.dram_tensor("A", [3, RPC], F32R, kind="ExternalInput").ap()
    op3_ap = nc.dram_tensor("op3", [3, RPC], F32, kind="ExternalInput").ap()
    p_ap = nc.dram_tensor("PAR", [128, PAR_W], F32R, kind="ExternalInput").ap()
    out_ap = nc.dram_tensor("out", [1, RPC], F32, kind="ExternalOutput").ap()

    with tile.TileContext(nc) as tc:
        with tc.tile_pool(name="const", bufs=1) as cpool, \
             tc.tile_pool(name="work", bufs=2) as wpool, \
             tc.tile_pool(name="gwork", bufs=1) as gpool, \
             tc.tile_pool(name="psum", bufs=2, space="PSUM") as ppool, \
             tc.tile_pool(name="psum_sm", bufs=3, space="PSUM") as pps, \
             tc.tile_pool(name="psum_t", bufs=1, space="PSUM") as ppt:
            _emit_body(tc, nc, A_ap, op3_ap, p_ap, out_ap, force_fallback,
                       emit_fallback, cpool, wpool, gpool, ppool, pps, ppt)
    nc.compile()
    return nc


def _emit_body(tc, nc, A_ap, op3_ap, p_ap, out_ap, force_fallback,
               emit_fallback, cpool, wpool, gpool, ppool, pps, ppt):
    # ---- constants: one packed param DMA, per-param slice views ----
    par = cpool.tile([128, PAR_W], F32R, tag="par")
    nc.sync.dma_start(par[:], p_ap[:])
    pc = {}
    off = 0
    for k in PAR_ORDER:
        (rows, cols), dt = PARAM_SPECS[k]
        v = par[0:rows, off:off + cols]
        pc[k] = v if dt == F32R else v.bitcast(F32)
        off += cols
    ones = cpool.tile([128, 1], F32, tag="ones")       # column of ones
    nc.gpsimd.memset(ones[:], 1.0)
    onesr = cpool.tile([1, 128], F32, tag="onesr")     # f32 ones row
    nc.gpsimd.memset(onesr[:], 1.0)
    onesrr = cpool.tile([1, 64], F32R, tag="onesrr")   # f32r ones row (m1 bcast)
    nc.vector.tensor_copy(onesrr[:], onesr[:, 0:64])
    id128 = cpool.tile([128, 128], F32, tag="id128")
    make_identity(nc, id128[:])
    LG = cpool.tile([96, 512], F32, tag="LG")
    nc.gpsimd.memset(LG[:], -1e30)
    iota3 = cpool.tile([3, TILE], F32, tag="iota3")    # value = partition idx
    nc.gpsimd.iota(iota3[:], pattern=[[0, TILE]], base=0, channel_multiplier=1,
                   allow_small_or_imprecise_dtypes=True)
    out_sb = cpool.tile([1, RPC], F32, tag="out_sb")
    cms = cpool.tile([1, NT], F32, tag="cms")          # per-tile mode count
    tc.strict_bb_all_engine_barrier()

    # PE warm-up burst: back-to-back small matmuls trip the HAM clock gate
    # to 2.4 GHz before the steady loop (cold-state matmuls run at 1.2 GHz).
    wsc = ppool.tile([128, 64], F32, tag="big")
    for k in range(25):
        nc.tensor.matmul(wsc[:], par[:, 64:192].bitcast(F32), par[:, 0:64].bitcast(F32),
                         start=(k == 0), stop=(k == 24))
    nc.scalar.activation(cms[:, 0:1], wsc[0:1, 0:1], AF.Copy)

    # ---------------- happy path over tiles (2-stage software pipeline) ----------------
    def phase_a(i):
        """Encode, route, stage weights for tile i. Returns state for phase_b."""
        ch = ts(i, TILE)
        at = wpool.tile([3, TILE], F32R, tag="at")
        nc.sync.dma_start(at[:], A_ap[:, ch])
        opt = wpool.tile([3, TILE], F32, tag="opt")
        nc.sync.dma_start(opt[:], op3_ap[:, ch])

        # encoders -> per-slot pre-state z (64) stacked on free dim
        psA = ppool.tile([128, TILE], F32, tag="big")
        nc.tensor.matmul(psA[:], pc["LA"], at[:], start=True, stop=True)
        ra = wpool.tile([128, TILE], F32R, tag="ra")
        nc.scalar.activation(ra[:], psA[:], AF.Relu)
        psZ = ppool.tile([128, TILE], F32, tag="big")
        nc.tensor.matmul(psZ[:], pc["LZ"], ra[:], start=True, stop=True)
        oh3 = wpool.tile([3, TILE], F32R, tag="oh3")
        nc.vector.tensor_tensor(oh3[:], iota3[:], opt[:], ALU.is_equal)
        psOP = ppool.tile([64, TILE], F32, tag="med")
        nc.tensor.matmul(psOP[:], pc["LOP"], oh3[:], start=True, stop=True)

        zs = wpool.tile([64, 3 * TILE], F32R, tag="zs")
        nc.scalar.activation(zs[:, 0:TILE], psZ[0:64, :], AF.Relu, bias=pc["czz"][0:64, :])
        nc.scalar.activation(zs[:, TILE:2 * TILE], psZ[64:128, :], AF.Relu, bias=pc["czz"][64:128, :])
        nc.scalar.activation(zs[:, 2 * TILE:3 * TILE], psOP[:], AF.Relu)

        # heads: logits into padded 96-row LG (pads stay -1e30); d via z-delta
        psL0 = pps.tile([8, TILE], F32, tag="sm")
        psL1 = pps.tile([8, TILE], F32, tag="sm")
        psHc = pps.tile([8, TILE], F32, tag="sm")
        nc.tensor.matmul(psL0[:], pc["LH"][:, 0:8], zs[:, 0:TILE], start=True, stop=True)
        nc.tensor.matmul(psL1[:], pc["LH"][:, 0:8], zs[:, TILE:2 * TILE], start=True, stop=True)
        nc.tensor.matmul(psHc[:], pc["LH"][:, 0:8], zs[:, 2 * TILE:3 * TILE], start=True, stop=True)
        nc.scalar.activation(LG[0:8, :], psL0[:], AF.Identity, bias=pc["c1p"])
        nc.scalar.activation(LG[32:40, :], psL1[:], AF.Identity, bias=pc["c1p"])
        nc.scalar.activation(LG[64:72, :], psHc[:], AF.Identity, bias=pc["c1p"])
        vd = wpool.tile([64, TILE], F32R, tag="vd")
        nc.vector.tensor_tensor(vd[:], zs[:, TILE:2 * TILE].bitcast(F32),
                                zs[:, 0:TILE].bitcast(F32), ALU.subtract)
        psSD = pps.tile([8, TILE], F32, tag="sm")
        nc.tensor.matmul(psSD[:], pc["LH"][:, 8:16], vd[:], start=True, stop=True)
        sd = wpool.tile([8, TILE], F32R, tag="sd")
        nc.vector.tensor_copy(sd[:], psSD[:])

        # per-row flat argmax via transpose + free-dim max (96-bin padded)
        psT = ppt.tile([128, 384], F32, tag="smT")
        for j in range(4):
            nc.tensor.transpose(psT[:, ts(j, 96)], LG[:, ts(j, 128)], id128[0:96, 0:96])
        mx = wpool.tile([128, 32], F32, tag="mx")
        ohn = wpool.tile([128, 384], F32, tag="ohn")
        for j in range(4):
            nc.vector.max(mx[:, ts(j, 8)], psT[:, ts(j, 96)])
            nc.vector.tensor_scalar(ohn[:, ts(j, 96)], psT[:, ts(j, 96)],
                                    mx[:, 8 * j:8 * j + 1], None, ALU.is_equal)
        psC = pps.tile([96, 1], F32, tag="sm")
        for j in range(4):
            nc.tensor.matmul(psC[:], ohn[:, ts(j, 96)], ones[:], start=(j == 0), stop=(j == 3))
        cnt = wpool.tile([96, 1], F32, tag="cnt")
        nc.scalar.activation(cnt[:], psC[:], AF.Identity)
        psCT = pps.tile([1, 96], F32, tag="sm")
        nc.tensor.transpose(psCT[:], cnt[:], id128[0:96, 0:96])
        cntr = wpool.tile([1, 96], F32, tag="cntr")
        nc.scalar.activation(cntr[:], psCT[:], AF.Identity)
        cmx = wpool.tile([1, 8], F32, tag="cmx")
        nc.vector.max(cmx[:], cntr[:])
        nc.vector.tensor_copy(cms[:, i:i + 1], cmx[:, 0:1])

        # mode one-hot over the 96 flat bins -> expert sel + offsets
        psCMB = pps.tile([96, 1], F32, tag="sm")
        nc.tensor.matmul(psCMB[:], onesr[:, 0:96], cmx[:, 0:1], start=True, stop=True)
        oh24 = wpool.tile([96, 1], F32, tag="oh24")
        nc.vector.tensor_tensor(oh24[:], cnt[:], psCMB[:], ALU.is_equal)
        psS8 = pps.tile([8, 1], F32, tag="sm")
        nc.tensor.matmul(psS8[:], pc["S8"], oh24[:], start=True, stop=True)
        sel8 = wpool.tile([8, 1], F32R, tag="sel8")
        nc.scalar.activation(sel8[:], psS8[:], AF.Identity)
        psE2 = pps.tile([1, 1], F32, tag="sm")
        nc.tensor.matmul(psE2[:], pc["CE"], oh24[:], start=True, stop=True)
        psP5 = pps.tile([1, 1], F32, tag="sm")
        nc.tensor.matmul(psP5[:], pc["CP"], oh24[:], start=True, stop=True)
        eoff_i = wpool.tile([1, 1], I32, tag="eoff_i")
        nc.vector.tensor_scalar(eoff_i[:], psE2[:], float(WE_OFF + 1400), None, ALU.min)
        poff_i = wpool.tile([1, 1], I32, tag="poff_i")
        nc.vector.tensor_scalar(poff_i[:], psP5[:], 1024.0, None, ALU.min)
        rv_e = nc.values_load(eoff_i[0:1, 0:1], engines=(mybir.EngineType.DVE,),
                              min_val=WE_OFF, max_val=WE_OFF + 1400,
                              skip_runtime_bounds_check=True)
        rv_p = nc.values_load(poff_i[0:1, 0:1], engines=(mybir.EngineType.DVE,),
                              min_val=0, max_val=1024, skip_runtime_bounds_check=True)
        stg = wpool.tile([128, 200], F32R, tag="stg")
        nc.vector.tensor_copy(stg[:], par[:, ds(rv_e, 200)])
        vblk = wpool.tile([128, TILE], F32R, tag="vblk")
        nc.vector.tensor_copy(vblk[0:64, :], zs[:, ds(rv_p, TILE)])
        return dict(i=i, zs=zs, vd=vd, sd=sd, sel8=sel8, stg=stg, vblk=vblk)

    def phase_b1(st):
        """Mask chain: contextual-slot choice -> vblk rows 64:128 for tile st['i']."""
        zs, vd, sd, sel8, vblk = (st[k] for k in ("zs", "vd", "sd", "sel8", "vblk"))
        psDS = pps.tile([1, TILE], F32, tag="sm")
        nc.tensor.matmul(psDS[:], sel8[:], sd[:], start=True, stop=True)
        m1 = wpool.tile([1, TILE], F32R, tag="m1")
        nc.vector.tensor_scalar(m1[:], psDS[:], 0.0, None, ALU.is_gt)
        psM1B = ppool.tile([64, TILE], F32, tag="med")
        nc.tensor.matmul(psM1B[:], onesrr[:], m1[:], start=True, stop=True)
        vm = wpool.tile([64, TILE], F32, tag="vm")
        nc.vector.tensor_tensor(vm[:], vd[:].bitcast(F32), psM1B[:], ALU.mult)
        nc.vector.tensor_tensor(vblk[64:128, :], vm[:], zs[:, 0:TILE].bitcast(F32), ALU.add)
        return st

    def phase_b2(st):
        """Expert MLP + folded decoder for tile st['i']."""
        i, stg, vblk = st["i"], st["stg"], st["vblk"]
        ch = ts(i, TILE)
        psE = ppool.tile([128, TILE], F32, tag="big")
        nc.tensor.matmul(psE[:], stg[:, 0:128], vblk[:], start=True, stop=True)
        he = wpool.tile([128, TILE], F32R, tag="he")
        nc.scalar.activation(he[:], psE[:], AF.Relu, bias=stg[:, 192:193].bitcast(F32))
        psO = ppool.tile([64, TILE], F32, tag="med")
        nc.tensor.matmul(psO[:], stg[:, 128:192], he[:], start=True, stop=True)
        ho = wpool.tile([64, TILE], F32R, tag="ho")
        nc.scalar.activation(ho[:], psO[:], AF.Relu, bias=stg[0:64, 193:194].bitcast(F32))
        psF = pps.tile([1, TILE], F32, tag="sm")
        nc.tensor.matmul(psF[:], pc["LF"], ho[:], start=True, stop=True)
        nc.scalar.activation(out_sb[:, ch], psF[:], AF.Identity, bias=pc["deb2"][0:1, 0:1])

    prev = None
    for i in range(NT):
        b1 = phase_b1(prev) if prev is not None else None
        st = phase_a(i)
        if b1 is not None:
            phase_b2(b1)
        prev = st
    phase_b2(phase_b1(prev))

    # ---------------- deviance check + deferred general path ----------------
    cmn = wpool.tile([1, 1], F32, tag="cmn")
    nc.vector.tensor_reduce(cmn[:], cms[:], AX.X, ALU.min)
    cmn_i = wpool.tile([1, 1], I32, tag="cmn_i")
    nc.vector.tensor_copy(cmn_i[:], cmn[:])
    rv_min = nc.values_load(cmn_i[0:1, 0:1], min_val=0, max_val=TILE,
                            skip_runtime_bounds_check=True)

    if emit_fallback:
        thresh = TILE + 1 if force_fallback else TILE
        with tc.If(rv_min < thresh):
            _emit_general(tc, nc, A_ap, op3_ap, pc, out_sb,
                          cpool, gpool, ppool, pps, ones, onesr, id128, iota3)

    nc.sync.dma_start(out_ap[:], out_sb[:])


def _emit_general(tc, nc, A_ap, op3_ap, pc, out_sb, cpool, wpool, ppool, pps,
                  ones, onesr, id128, iota3):
    """Fully general masked all-expert recompute of the whole core (plain f32).

    Runs only if some tile had a row disagreeing with its mode (never on the
    observed data). Overwrites out_sb entirely.
    """
    iota8 = cpool.tile([8, TILE], F32, tag="iota8")
    nc.gpsimd.iota(iota8[:], pattern=[[0, TILE]], base=0, channel_multiplier=1,
                   allow_small_or_imprecise_dtypes=True)
    for i in range(NT):
        ch = ts(i, TILE)
        at = wpool.tile([3, TILE], F32, tag="g_at")
        nc.sync.dma_start(at[:], A_ap[:, ch].bitcast(F32))
        opt = wpool.tile([3, TILE], F32, tag="g_opt")
        nc.sync.dma_start(opt[:], op3_ap[:, ch])

        psA = ppool.tile([128, TILE], F32, tag="big")
        nc.tensor.matmul(psA[:], pc["LA"].bitcast(F32), at[:], start=True, stop=True)
        ra = wpool.tile([128, TILE], F32, tag="g_ra")
        nc.scalar.activation(ra[:], psA[:], AF.Relu)
        psZ = ppool.tile([128, TILE], F32, tag="big")
        nc.tensor.matmul(psZ[:], pc["LZ"].bitcast(F32), ra[:], start=True, stop=True)
        oh3 = wpool.tile([3, TILE], F32, tag="g_oh3")
        nc.vector.tensor_tensor(oh3[:], iota3[:], opt[:], ALU.is_equal)
        psOP = ppool.tile([64, TILE], F32, tag="med")
        nc.tensor.matmul(psOP[:], pc["LOP"].bitcast(F32), oh3[:], start=True, stop=True)
        zs = wpool.tile([64, 3 * TILE], F32, tag="g_zs")
        nc.scalar.activation(zs[:, 0:TILE], psZ[0:64, :], AF.Relu, bias=pc["czz"][0:64, :])
        nc.scalar.activation(zs[:, TILE:2 * TILE], psZ[64:128, :], AF.Relu, bias=pc["czz"][64:128, :])
        nc.scalar.activation(zs[:, 2 * TILE:3 * TILE], psOP[:], AF.Relu)

        LHf = pc["LH"][:].bitcast(F32)
        psHa = ppool.tile([64, TILE], F32, tag="big")
        psHb = ppool.tile([64, TILE], F32, tag="med")
        psHc = pps.tile([8, TILE], F32, tag="sm")
        nc.tensor.matmul(psHa[0:8, :], LHf[:, 0:8], zs[:, 0:TILE], start=True, stop=True)
        nc.tensor.matmul(psHa[32:40, :], LHf[:, 8:16], zs[:, 0:TILE], start=True, stop=True)
        nc.tensor.matmul(psHb[0:8, :], LHf[:, 0:8], zs[:, TILE:2 * TILE], start=True, stop=True)
        nc.tensor.matmul(psHb[32:40, :], LHf[:, 8:16], zs[:, TILE:2 * TILE], start=True, stop=True)
        nc.tensor.matmul(psHc[:], LHf[:, 0:8], zs[:, 2 * TILE:3 * TILE], start=True, stop=True)
        l0 = wpool.tile([8, TILE], F32, tag="g_l0")
        l1 = wpool.tile([8, TILE], F32, tag="g_l1")
        l2 = wpool.tile([8, TILE], F32, tag="g_l2")
        nc.scalar.activation(l0[:], psHa[0:8, :], AF.Identity, bias=pc["c1p"])
        nc.scalar.activation(l1[:], psHb[0:8, :], AF.Identity, bias=pc["c1p"])
        nc.scalar.activation(l2[:], psHc[:], AF.Identity, bias=pc["c1p"])
        a0s = wpool.tile([8, TILE], F32, tag="g_a0s")
        nc.scalar.activation(a0s[:], psHa[32:40, :], AF.Copy)
        sd = wpool.tile([8, TILE], F32, tag="g_sd")
        nc.vector.tensor_tensor(sd[:], psHb[32:40, :], a0s[:], ALU.subtract)

        # per-row argmax -> slot/rule rows in transposed layout
        psT = pps.tile([128, 96], F32, tag="sm")
        for j in range(4):
            for t, lt in enumerate((l0, l1, l2)):
                nc.tensor.transpose(psT[:, 24 * j + 8 * t:24 * j + 8 * (t + 1)],
                                    lt[:, ts(j, 128)], id128[0:8, 0:8])
        vT = wpool.tile([128, 96], F32, tag="g_vT")
        nc.scalar.activation(vT[:], psT[:], AF.Copy)
        tn = wpool.tile([128, 2], F32, tag="g_tn")      # per-chunk [t, n] cols
        tr_t = wpool.tile([1, TILE], F32, tag="g_trt")  # slot row
        tr_n = wpool.tile([1, TILE], F32, tag="g_trn")  # rule row
        for j in range(4):
            mxj = wpool.tile([128, 8], F32, tag="g_mxj")
            mij = wpool.tile([128, 8], U32, tag="g_mij")
            nc.vector.max_with_indices(mxj[:], mij[:], vT[:, 24 * j:24 * (j + 1)])
            jf = wpool.tile([128, 1], F32, tag="g_jf")
            nc.vector.tensor_copy(jf[:], mij[:, 0:1])
            # t = 1{j>=8} + 1{j>=16};  n = j - 8t
            t1 = wpool.tile([128, 1], F32, tag="g_t1")
            nc.vector.tensor_scalar(t1[:], jf[:], 7.5, None, ALU.is_gt)
            t2 = wpool.tile([128, 1], F32, tag="g_t2")
            nc.vector.tensor_scalar(t2[:], jf[:], 15.5, None, ALU.is_gt)
            nc.vector.tensor_tensor(tn[:, 0:1], t1[:], t2[:], ALU.add)
            t8 = wpool.tile([128, 1], F32, tag="g_t8")
            nc.vector.tensor_scalar(t8[:], tn[:, 0:1], -8.0, None, ALU.mult)
            nc.vector.tensor_tensor(tn[:, 1:2], jf[:], t8[:], ALU.add)
            psTT = pps.tile([1, 128], F32, tag="sm")
            nc.tensor.transpose(psTT[:], tn[:, 0:1], id128[:])
            nc.scalar.activation(tr_t[:, ts(j, 128)], psTT[:], AF.Copy)
            psTN = pps.tile([1, 128], F32, tag="sm")
            nc.tensor.transpose(psTN[:], tn[:, 1:2], id128[:])
            nc.scalar.activation(tr_n[:, ts(j, 128)], psTN[:], AF.Copy)

        # rule one-hot (transposed) + slot row broadcast to 64 partitions
        psNB = pps.tile([8, TILE], F32, tag="sm")
        nc.tensor.matmul(psNB[:], onesr[:, 0:8], tr_n[:], start=True, stop=True)
        ohT8 = wpool.tile([8, TILE], F32, tag="g_ohT8")
        nc.vector.tensor_tensor(ohT8[:], iota8[:], psNB[:], ALU.is_equal)
        psTB = ppool.tile([64, TILE], F32, tag="med")
        nc.tensor.matmul(psTB[:], onesr[:, 0:64], tr_t[:], start=True, stop=True)
        tslot = wpool.tile([64, TILE], F32, tag="g_tslot")
        nc.scalar.activation(tslot[:], psTB[:], AF.Copy)

        # zp = sum_t zs_t * 1{slot==t}
        zp = wpool.tile([64, TILE], F32, tag="g_zp")
        acc = wpool.tile([64, TILE], F32, tag="g_acc")
        ohm = wpool.tile([64, TILE], F32, tag="g_ohm")
        for t in range(3):
            nc.vector.tensor_scalar(ohm[:], tslot[:], float(t), None, ALU.is_equal)
            dst = zp if t == 0 else acc
            nc.vector.tensor_tensor(dst[:], zs[:, t * TILE:(t + 1) * TILE], ohm[:], ALU.mult)
            if t > 0:
                nc.vector.tensor_tensor(zp[:], zp[:], acc[:], ALU.add)
        sdm = wpool.tile([8, TILE], F32, tag="g_sdm")
        nc.vector.tensor_tensor(sdm[:], sd[:], ohT8[:], ALU.mult)
        psDR = pps.tile([1, TILE], F32, tag="sm")
        nc.tensor.matmul(psDR[:], ones[0:8, :], sdm[:], start=True, stop=True)
        m1 = wpool.tile([1, TILE], F32, tag="g_m1")
        nc.vector.tensor_scalar(m1[:], psDR[:], 0.0, None, ALU.is_gt)
        psM1B = ppool.tile([64, TILE], F32, tag="med")
        nc.tensor.matmul(psM1B[:], onesr[:, 0:64], m1[:], start=True, stop=True)
        m1bi = wpool.tile([64, TILE], I32, tag="g_m1bi")
        nc.vector.tensor_copy(m1bi[:], psM1B[:])
        vc = wpool.tile([64, TILE], F32, tag="g_vc")
        nc.vector.select(vc[:], m1bi[:], zs[:, TILE:2 * TILE], zs[:, 0:TILE])

        # all experts, masked combine
        oacc = wpool.tile([1, TILE], F32, tag="g_oacc")
        me = wpool.tile([1, TILE], F32, tag="g_me")
        fm = wpool.tile([1, TILE], F32, tag="g_fm")
        for e in range(NR):
            psE = ppool.tile([128, TILE], F32, tag="big")
            nc.tensor.matmul(psE[:], pc["W1a"][:, ts(e, 128)], zp[:], start=True, stop=False)
            nc.tensor.matmul(psE[:], pc["W1b"][:, ts(e, 128)], vc[:], start=False, stop=True)
            he = wpool.tile([128, TILE], F32, tag="g_he")
            nc.scalar.activation(he[:], psE[:], AF.Relu, bias=pc["b1e"][:, e:e + 1])
            psO = ppool.tile([64, TILE], F32, tag="med")
            nc.tensor.matmul(psO[:], pc["W2f"][:, ts(e, 64)], he[:], start=True, stop=True)
            ho = wpool.tile([64, TILE], F32, tag="g_ho")
            nc.scalar.activation(ho[:], psO[:], AF.Relu, bias=pc["b2e"][:, e:e + 1])
            psF = pps.tile([1, TILE], F32, tag="sm")
            nc.tensor.matmul(psF[:], pc["LF"].bitcast(F32), ho[:], start=True, stop=True)
            fe = wpool.tile([1, TILE], F32, tag="g_fe")
            nc.scalar.activation(fe[:], psF[:], AF.Identity, bias=pc["deb2"][0:1, 0:1])
            nc.vector.tensor_scalar(me[:], tr_n[:], float(e), None, ALU.is_equal)
            nc.vector.tensor_tensor(fm[:], fe[:], me[:], ALU.mult)
            if e == 0:
                nc.vector.tensor_copy(oacc[:], fm[:])
            else:
                nc.vector.tensor_tensor(oacc[:], oacc[:], fm[:], ALU.add)
        nc.vector.tensor_copy(out_sb[:, ch], oacc[:])


def _shard_inputs(operand1, operand2, operator, PAR):
    o1 = np.asarray(operand1, np.float32)
    o2 = np.asarray(operand2, np.float32)
    opf = np.asarray(operator).astype(np.float32)
    in_maps = []
    for c in range(NCORES):
        sl = slice(c * RPC, (c + 1) * RPC)
        A = np.stack([o1[sl], np.ones(RPC, np.float32), o2[sl]])
        op3 = np.repeat(opf[sl][None, :], 3, axis=0)
        in_maps.append({"A": A, "op3": op3, "PAR": PAR})
    return in_maps


_NC_CACHE = {}


def kernel(operand1, operand2, operator, params):
    P = {k: np.asarray(v) for k, v in params.items()}
    PAR = _fold_params(P)
    in_maps = _shard_inputs(operand1, operand2, operator, PAR)
    if "nc" not in _NC_CACHE:
        _NC_CACHE["nc"] = build_nc()
    res = run_bass_kernel_spmd(_NC_CACHE["nc"], in_maps, core_ids=list(range(NCORES)))
    out = np.concatenate([res.results[c]["out"].reshape(-1) for c in range(NCORES)])
    return out.astype(np.float32)


if __name__ == "__main__":
    d = np.load("/tmp/inputs.npz")
    params = {k: d[k] for k in d.files if k not in ("operand1", "operand2", "operator")}
    out = kernel(d["operand1"], d["operand2"], d["operator"], params)
    exp = np.load("/tmp/expected.npy")
    rel = np.linalg.norm(out - exp) / np.linalg.norm(exp)
    print("Relative error:", rel)


# revision 49
# speedup vs baseline: 1.0851x; 1.0553x over previous
"""Trainium2 Bass kernel for nn_ArithmeticNps (moe_routing).

Strategy
--------
Data-parallel over batch: each of the 8 cores handles 8192 rows; params are
replicated (tiny). All activations live in transposed layout (features on
partitions, batch on the free dim), so the whole MLP chain runs without
transposes; weights fold through adjacent linear layers on the host:

  - state=0 halves se_W1; encoder L2 folds into the state encoder
    (E' = eo_W2 @ se_W1[cv:]), so each slot's pre-relu state-encoding `z`
    (64-dim) is one matmul away from the wrapped inputs.
  - selector key projections fold through se_W2 (M1', M2'), so rule logits
    and contextual-slot logits come straight from relu(z).
  - expert W1 folds through se_W2 (per-expert 128-dim hidden from the two
    selected 64-dim relu(z) vectors), expert W2 folds through the decoder
    de_W1.

Routing: per 512-row tile, the flat argmax over the 24 (rule, slot) logits
is computed per row via PE transpose + DVE max; the tile's mode (dominant
flat index) picks the expert weights/primary slot through dynamic SBUF
slices (offsets come from tiny constant-table matmuls against the mode
one-hot). Rows disagreeing with the mode are counted; if any tile has a
deviant row, one deferred fully-general (masked, all-expert) pass recomputes
the whole core's output. On the observed data distribution every row agrees,
so the general pass never executes.

Heavy matmuls run as float32r (full-rate fp32 PE mode); walrus requires
f32r operands to be *produced* as f32r, so matmul-feeding DRAM params and
compute outputs are declared float32r end-to-end. The general path sticks
to plain f32 via bitcasts (it never runs; speed irrelevant).
"""

import numpy as np

import concourse.bass as bass
import concourse.mybir as mybir
import concourse.tile as tile
from concourse import bacc
from concourse.bass import ds, ts
from concourse.bass_utils import run_bass_kernel_spmd
from concourse.masks import make_identity

F32 = mybir.dt.float32
F32R = mybir.dt.float32r
I32 = mybir.dt.int32
U32 = mybir.dt.uint32
AF = mybir.ActivationFunctionType
ALU = mybir.AluOpType
AX = mybir.AxisListType

B = 65536
NCORES = 8
RPC = B // NCORES          # rows per core
TILE = 512
NT = RPC // TILE           # tiles per core
CV = 256
NR = 8


def _fold_params(P):
    """Host-side weight folding. All O(param) numpy math."""
    f = np.float32
    g = {k: np.asarray(v, f) for k, v in P.items()}
    se_lo = g["se_W1"][CV:]                  # (256, 64)  state=0 kills the top half

    # wrapped-operand pre-activations from A = [v1; 1; v2]
    LA = np.zeros((3, 128), f)
    LA[0, 0:64] = g["eo_W1"][0]
    LA[1, 0:64] = g["eo_b1"]
    LA[1, 64:128] = g["eo_W1"][1] + g["eo_b1"]
    LA[2, 64:128] = g["eo_W1"][0]

    E = (g["eo_W2"] @ se_lo).astype(f)       # (64, 64)
    cz = (g["eo_b2"] @ se_lo + g["se_b1"]).astype(f)
    LZ = np.zeros((128, 128), f)
    LZ[0:64, 0:64] = E
    LZ[64:128, 64:128] = E
    czz = np.concatenate([cz, cz]).astype(f)[:, None]              # (128, 1)

    ope_enc = (np.maximum(g["er_W1"] + g["er_b1"], 0) @ g["er_W2"] + g["er_b2"])
    LOP = (ope_enc @ se_lo + g["se_b1"]).astype(f)                 # (3, 64)

    read1 = (g["body"] @ g["s1_Wq"] + g["s1_bq"]).astype(f)        # (8, 32)
    M1 = (g["se_W2"] @ g["s1_Wk"] @ read1.T / np.sqrt(32.0)).astype(f)
    c1p = (((g["se_b2"] @ g["s1_Wk"] + g["s1_bk"]) @ read1.T) / np.sqrt(32.0)).astype(f)[:, None]
    q2t = (g["body"] @ g["s2_Wq"] + g["s2_bq"]).astype(f)          # (8, 16)
    M2 = (g["se_W2"] @ g["s2_Wk"] @ q2t.T / np.sqrt(16.0)).astype(f)
    LH = np.concatenate([M1, M2], axis=1).astype(f)                # (64, 16)

    rh_W1, rh_b1 = g["rh_W1"], g["rh_b1"]
    rh_W2, rh_b2 = g["rh_W2"], g["rh_b2"]
    de_W1, de_b1 = g["de_W1"], g["de_b1"]
    W1a = np.zeros((64, NR * 128), f)
    W1b = np.zeros((64, NR * 128), f)
    b1e = np.zeros((128, NR), f)
    W2f = np.zeros((128, NR * 64), f)
    b2e = np.zeros((64, NR), f)
    WE = np.zeros((128, NR * 200), f)
    for e in range(NR):
        W1a[:, e * 128:(e + 1) * 128] = g["se_W2"] @ rh_W1[e][:CV]
        W1b[:, e * 128:(e + 1) * 128] = g["se_W2"] @ rh_W1[e][CV:]
        b1e[:, e] = rh_b1[e] + g["se_b2"] @ rh_W1[e][:CV] + g["se_b2"] @ rh_W1[e][CV:]
        W2f[:, e * 64:(e + 1) * 64] = rh_W2[e] @ de_W1
        b2e[:, e] = rh_b2[e] @ de_W1 + de_b1
        o = e * 200
        WE[0:64, o:o + 128] = W1a[:, e * 128:(e + 1) * 128]
        WE[64:128, o:o + 128] = W1b[:, e * 128:(e + 1) * 128]
        WE[:, o + 128:o + 192] = W2f[:, e * 64:(e + 1) * 64]
        WE[:, o + 192] = b1e[:, e]
        WE[0:64, o + 193] = b2e[:, e]
    LF = g["de_W2"].astype(f)                                      # (64, 1)
    deb2 = np.array([[g["de_b2"][0]]], f)

    # 96-bin flat index: j = 32*slot + rule (rows 8..31 of each block are pads)
    S8 = np.zeros((96, NR), f)
    CE = np.zeros((96, 1), f)
    CP = np.zeros((96, 1), f)
    for t in range(3):
        for n in range(8):
            j = 32 * t + n
            S8[j, n] = 1.0
            CE[j, 0] = WE_OFF + n * 200
            CP[j, 0] = t * 512

    parts = dict(LA=LA, LZ=LZ, czz=czz, LOP=LOP, LH=LH, c1p=c1p,
                 W1a=W1a, W1b=W1b, b1e=b1e, W2f=W2f, b2e=b2e, LF=LF,
                 deb2=deb2, S8=S8, WE=WE, CE=CE, CP=CP)
    W = sum(v.shape[1] for v in parts.values())
    PAR = np.zeros((128, W), f)
    off = 0
    for k in PAR_ORDER:
        v = parts[k]
        PAR[:v.shape[0], off:off + v.shape[1]] = v
        off += v.shape[1]
    return PAR


# name -> (shape, dtype). f32r for anything feeding an f32r matmul.
# All packed into one (128, W) f32r DRAM tensor "PAR" (single DMA).
PARAM_SPECS = dict(
    LA=((3, 128), F32R), LZ=((128, 128), F32R), LOP=((3, 64), F32R),
    LH=((64, 16), F32R), WE=((128, NR * 200), F32R), LF=((64, 1), F32R),
    S8=((96, NR), F32), CE=((96, 1), F32), CP=((96, 1), F32),
    czz=((128, 1), F32), c1p=((8, 1), F32), deb2=((1, 1), F32),
    W1a=((64, NR * 128), F32), W1b=((64, NR * 128), F32),
    b1e=((128, NR), F32), W2f=((128, NR * 64), F32), b2e=((64, NR), F32),
)
PAR_ORDER = list(PARAM_SPECS.keys())
PAR_W = sum(shp[1] for shp, _ in PARAM_SPECS.values())
WE_OFF = sum(PARAM_SPECS[k][0][1] for k in PAR_ORDER[:PAR_ORDER.index("WE")])


def build_nc(force_fallback=False, emit_fallback=True):
    nc = bacc.Bacc("TRN2", target_bir_lowering=False, debug=False)
    A_ap = nc.dram_tensor("A", [3, RPC], F32R, kind="ExternalInput").ap()
    op3_ap = nc.dram_tensor("op3", [3, RPC], F32, kind="ExternalInput").ap()
    p_ap = nc.dram_tensor("PAR", [128, PAR_W], F32R, kind="ExternalInput").ap()
    out_ap = nc.dram_tensor("out", [1, RPC], F32, kind="ExternalOutput").ap()

    with tile.TileContext(nc) as tc:
        with tc.tile_pool(name="const", bufs=1) as cpool, \
             tc.tile_pool(name="work", bufs=2) as wpool, \
             tc.tile_pool(name="gwork", bufs=1) as gpool, \
             tc.tile_pool(name="psum", bufs=2, space="PSUM") as ppool, \
             tc.tile_pool(name="psum_sm", bufs=4, space="PSUM") as pps:
            ppt = pps
            _emit_body(tc, nc, A_ap, op3_ap, p_ap, out_ap, force_fallback,
                       emit_fallback, cpool, wpool, gpool, ppool, pps, ppt)
    nc.compile()
    return nc


def _emit_body(tc, nc, A_ap, op3_ap, p_ap, out_ap, force_fallback,
               emit_fallback, cpool, wpool, gpool, ppool, pps, ppt):
    # ---- constants: one packed param DMA, per-param slice views ----
    par = cpool.tile([128, PAR_W], F32R, tag="par")
    nc.sync.dma_start(par[:], p_ap[:])
    pc = {}
    off = 0
    for k in PAR_ORDER:
        (rows, cols), dt = PARAM_SPECS[k]
        v = par[0:rows, off:off + cols]
        pc[k] = v if dt == F32R else v.bitcast(F32)
        off += cols
    ones = cpool.tile([128, 1], F32, tag="ones")       # column of ones
    nc.gpsimd.memset(ones[:], 1.0)
    onesr = cpool.tile([1, 128], F32, tag="onesr")     # f32 ones row
    nc.gpsimd.memset(onesr[:], 1.0)
    onesrr = cpool.tile([1, 64], F32R, tag="onesrr")   # f32r ones row (m1 bcast)
    nc.vector.tensor_copy(onesrr[:], onesr[:, 0:64])
    id128 = cpool.tile([128, 128], F32, tag="id128")
    make_identity(nc, id128[:])
    LG = cpool.tile([96, 512], F32, tag="LG")
    nc.gpsimd.memset(LG[:], -1e30)
    iota3 = cpool.tile([3, TILE], F32, tag="iota3")    # value = partition idx
    nc.gpsimd.iota(iota3[:], pattern=[[0, TILE]], base=0, channel_multiplier=1,
                   allow_small_or_imprecise_dtypes=True)
    out_sb = cpool.tile([1, RPC], F32, tag="out_sb")
    cms = cpool.tile([1, NT], F32, tag="cms")          # per-tile mode count
    tc.strict_bb_all_engine_barrier()

    # ---------------- happy path over tiles (2-stage software pipeline) ----------------
    def phase_a(i):
        """Encode, route, stage weights for tile i. Returns state for phase_b."""
        ch = ts(i, TILE)
        at = wpool.tile([3, TILE], F32R, tag="at")
        nc.sync.dma_start(at[:], A_ap[:, ch])
        opt = wpool.tile([3, TILE], F32, tag="opt")
        nc.sync.dma_start(opt[:], op3_ap[:, ch])

        # encoders -> per-slot pre-state z (64) stacked on free dim
        psA = ppool.tile([128, TILE], F32, tag="big")
        nc.tensor.matmul(psA[:], pc["LA"], at[:], start=True, stop=True)
        ra = wpool.tile([128, TILE], F32R, tag="ra")
        nc.scalar.activation(ra[:], psA[:], AF.Relu)
        psZ = ppool.tile([128, TILE], F32, tag="big")
        nc.tensor.matmul(psZ[:], pc["LZ"], ra[:], start=True, stop=True)
        oh3 = wpool.tile([3, TILE], F32R, tag="oh3")
        nc.vector.tensor_tensor(oh3[:], iota3[:], opt[:], ALU.is_equal)
        psOP = ppool.tile([64, TILE], F32, tag="med")
        nc.tensor.matmul(psOP[:], pc["LOP"], oh3[:], start=True, stop=True)

        zs = wpool.tile([64, 3 * TILE], F32R, tag="zs")
        nc.scalar.activation(zs[:, 0:TILE], psZ[0:64, :], AF.Relu, bias=pc["czz"][0:64, :])
        nc.scalar.activation(zs[:, TILE:2 * TILE], psZ[64:128, :], AF.Relu, bias=pc["czz"][64:128, :])
        nc.scalar.activation(zs[:, 2 * TILE:3 * TILE], psOP[:], AF.Relu)

        # heads: logits into padded 96-row LG (pads stay -1e30); d via z-delta
        psL0 = pps.tile([8, TILE], F32, tag="sm")
        psL1 = pps.tile([8, TILE], F32, tag="sm")
        psHc = pps.tile([8, TILE], F32, tag="sm")
        nc.tensor.matmul(psL0[:], pc["LH"][:, 0:8], zs[:, 0:TILE], start=True, stop=True)
        nc.tensor.matmul(psL1[:], pc["LH"][:, 0:8], zs[:, TILE:2 * TILE], start=True, stop=True)
        nc.tensor.matmul(psHc[:], pc["LH"][:, 0:8], zs[:, 2 * TILE:3 * TILE], start=True, stop=True)
        nc.scalar.activation(LG[0:8, :], psL0[:], AF.Identity, bias=pc["c1p"])
        nc.scalar.activation(LG[32:40, :], psL1[:], AF.Identity, bias=pc["c1p"])
        nc.scalar.activation(LG[64:72, :], psHc[:], AF.Identity, bias=pc["c1p"])
        vd = wpool.tile([64, TILE], F32R, tag="vd")
        nc.vector.tensor_tensor(vd[:], zs[:, TILE:2 * TILE].bitcast(F32),
                                zs[:, 0:TILE].bitcast(F32), ALU.subtract)
        psSD = pps.tile([8, TILE], F32, tag="sm")
        nc.tensor.matmul(psSD[:], pc["LH"][:, 8:16], vd[:], start=True, stop=True)
        sd = wpool.tile([8, TILE], F32R, tag="sd")
        nc.vector.tensor_copy(sd[:], psSD[:])

        # per-row flat argmax via transpose + free-dim max (96-bin padded)
        psT = pps.tile([128, 384], F32, tag="sm")
        for j in range(4):
            nc.tensor.transpose(psT[:, ts(j, 96)], LG[:, ts(j, 128)], id128[0:96, 0:96])
        mx = wpool.tile([128, 32], F32, tag="mx")
        ohn = wpool.tile([128, 384], F32, tag="ohn")
        for j in range(4):
            nc.vector.max(mx[:, ts(j, 8)], psT[:, ts(j, 96)])
            nc.vector.tensor_scalar(ohn[:, ts(j, 96)], psT[:, ts(j, 96)],
                                    mx[:, 8 * j:8 * j + 1], None, ALU.is_equal)
        psC = pps.tile([96, 1], F32, tag="sm")
        for j in range(4):
            nc.tensor.matmul(psC[:], ohn[:, ts(j, 96)], ones[:], start=(j == 0), stop=(j == 3))
        cnt = wpool.tile([96, 1], F32, tag="cnt")
        nc.scalar.activation(cnt[:], psC[:], AF.Identity)
        psCT = pps.tile([1, 96], F32, tag="sm")
        nc.tensor.transpose(psCT[:], cnt[:], id128[0:96, 0:96])
        cntr = wpool.tile([1, 96], F32, tag="cntr")
        nc.scalar.activation(cntr[:], psCT[:], AF.Identity)
        cmx = wpool.tile([1, 8], F32, tag="cmx")
        nc.vector.max(cmx[:], cntr[:])
        nc.vector.tensor_copy(cms[:, i:i + 1], cmx[:, 0:1])

        # mode one-hot over the 96 flat bins -> expert sel + offsets
        psCMB = pps.tile([96, 1], F32, tag="sm")
        nc.tensor.matmul(psCMB[:], onesr[:, 0:96], cmx[:, 0:1], start=True, stop=True)
        oh24 = wpool.tile([96, 1], F32, tag="oh24")
        nc.vector.tensor_tensor(oh24[:], cnt[:], psCMB[:], ALU.is_equal)
        psS8 = pps.tile([8, 1], F32, tag="sm")
        nc.tensor.matmul(psS8[:], pc["S8"], oh24[:], start=True, stop=True)
        sel8 = wpool.tile([8, 1], F32R, tag="sel8")
        nc.scalar.activation(sel8[:], psS8[:], AF.Identity)
        psE2 = pps.tile([1, 1], F32, tag="sm")
        nc.tensor.matmul(psE2[:], pc["CE"], oh24[:], start=True, stop=True)
        psP5 = pps.tile([1, 1], F32, tag="sm")
        nc.tensor.matmul(psP5[:], pc["CP"], oh24[:], start=True, stop=True)
        eoff_i = wpool.tile([1, 1], I32, tag="eoff_i")
        nc.vector.tensor_scalar(eoff_i[:], psE2[:], float(WE_OFF + 1400), None, ALU.min)
        poff_i = wpool.tile([1, 1], I32, tag="poff_i")
        nc.vector.tensor_scalar(poff_i[:], psP5[:], 1024.0, None, ALU.min)
        rv_e = nc.values_load(eoff_i[0:1, 0:1], engines=(mybir.EngineType.DVE,),
                              min_val=WE_OFF, max_val=WE_OFF + 1400,
                              skip_runtime_bounds_check=True)
        rv_p = nc.values_load(poff_i[0:1, 0:1], engines=(mybir.EngineType.DVE,),
                              min_val=0, max_val=1024, skip_runtime_bounds_check=True)
        stg = wpool.tile([128, 200], F32R, tag="stg")
        nc.vector.tensor_copy(stg[:], par[:, ds(rv_e, 200)])
        vblk = wpool.tile([128, TILE], F32R, tag="vblk")
        nc.vector.tensor_copy(vblk[0:64, :], zs[:, ds(rv_p, TILE)])
        return dict(i=i, zs=zs, vd=vd, sd=sd, sel8=sel8, stg=stg, vblk=vblk)

    def phase_b1(st):
        """Mask chain: contextual-slot choice -> vblk rows 64:128 for tile st['i']."""
        zs, vd, sd, sel8, vblk = (st[k] for k in ("zs", "vd", "sd", "sel8", "vblk"))
        psDS = pps.tile([1, TILE], F32, tag="sm")
        nc.tensor.matmul(psDS[:], sel8[:], sd[:], start=True, stop=True)
        m1 = wpool.tile([1, TILE], F32R, tag="m1")
        nc.vector.tensor_scalar(m1[:], psDS[:], 0.0, None, ALU.is_gt)
        psM1B = ppool.tile([64, TILE], F32, tag="med")
        nc.tensor.matmul(psM1B[:], onesrr[:], m1[:], start=True, stop=True)
        vm = wpool.tile([64, TILE], F32, tag="vm")
        nc.vector.tensor_tensor(vm[:], vd[:].bitcast(F32), psM1B[:], ALU.mult)
        nc.vector.tensor_tensor(vblk[64:128, :], vm[:], zs[:, 0:TILE].bitcast(F32), ALU.add)
        return st

    def phase_b2(st):
        """Expert MLP + folded decoder for tile st['i']."""
        i, stg, vblk = st["i"], st["stg"], st["vblk"]
        ch = ts(i, TILE)
        psE = ppool.tile([128, TILE], F32, tag="big")
        nc.tensor.matmul(psE[:], stg[:, 0:128], vblk[:], start=True, stop=True)
        he = wpool.tile([128, TILE], F32R, tag="he")
        nc.scalar.activation(he[:], psE[:], AF.Relu, bias=stg[:, 192:193].bitcast(F32))
        psO = ppool.tile([64, TILE], F32, tag="med")
        nc.tensor.matmul(psO[:], stg[:, 128:192], he[:], start=True, stop=True)
        ho = wpool.tile([64, TILE], F32R, tag="ho")
        nc.scalar.activation(ho[:], psO[:], AF.Relu, bias=stg[0:64, 193:194].bitcast(F32))
        psF = pps.tile([1, TILE], F32, tag="sm")
        nc.tensor.matmul(psF[:], pc["LF"], ho[:], start=True, stop=True)
        nc.scalar.activation(out_sb[:, ch], psF[:], AF.Identity, bias=pc["deb2"][0:1, 0:1])

    prev = None
    for i in range(NT):
        b1 = phase_b1(prev) if prev is not None else None
        st = phase_a(i)
        if b1 is not None:
            phase_b2(b1)
        prev = st
    phase_b2(phase_b1(prev))

    # ---------------- deviance check + deferred general path ----------------
    cmn = wpool.tile([1, 1], F32, tag="cmn")
    nc.vector.tensor_reduce(cmn[:], cms[:], AX.X, ALU.min)
    cmn_i = wpool.tile([1, 1], I32, tag="cmn_i")
    nc.vector.tensor_copy(cmn_i[:], cmn[:])
    rv_min = nc.values_load(cmn_i[0:1, 0:1], min_val=0, max_val=TILE,
                            skip_runtime_bounds_check=True)

    if emit_fallback:
        thresh = TILE + 1 if force_fallback else TILE
        with tc.If(rv_min < thresh):
            _emit_general(tc, nc, A_ap, op3_ap, pc, out_sb,
                          cpool, gpool, ppool, pps, ones, onesr, id128, iota3)

    nc.sync.dma_start(out_ap[:], out_sb[:])


def _emit_general(tc, nc, A_ap, op3_ap, pc, out_sb, cpool, wpool, ppool, pps,
                  ones, onesr, id128, iota3):
    """Fully general masked all-expert recompute of the whole core (plain f32).

    Runs only if some tile had a row disagreeing with its mode (never on the
    observed data). Overwrites out_sb entirely.
    """
    iota8 = cpool.tile([8, TILE], F32, tag="iota8")
    nc.gpsimd.iota(iota8[:], pattern=[[0, TILE]], base=0, channel_multiplier=1,
                   allow_small_or_imprecise_dtypes=True)
    for i in range(NT):
        ch = ts(i, TILE)
        at = wpool.tile([3, TILE], F32, tag="g_at")
        nc.sync.dma_start(at[:], A_ap[:, ch].bitcast(F32))
        opt = wpool.tile([3, TILE], F32, tag="g_opt")
        nc.sync.dma_start(opt[:], op3_ap[:, ch])

        psA = ppool.tile([128, TILE], F32, tag="big")
        nc.tensor.matmul(psA[:], pc["LA"].bitcast(F32), at[:], start=True, stop=True)
        ra = wpool.tile([128, TILE], F32, tag="g_ra")
        nc.scalar.activation(ra[:], psA[:], AF.Relu)
        psZ = ppool.tile([128, TILE], F32, tag="big")
        nc.tensor.matmul(psZ[:], pc["LZ"].bitcast(F32), ra[:], start=True, stop=True)
        oh3 = wpool.tile([3, TILE], F32, tag="g_oh3")
        nc.vector.tensor_tensor(oh3[:], iota3[:], opt[:], ALU.is_equal)
        psOP = ppool.tile([64, TILE], F32, tag="med")
        nc.tensor.matmul(psOP[:], pc["LOP"].bitcast(F32), oh3[:], start=True, stop=True)
        zs = wpool.tile([64, 3 * TILE], F32, tag="g_zs")
        nc.scalar.activation(zs[:, 0:TILE], psZ[0:64, :], AF.Relu, bias=pc["czz"][0:64, :])
        nc.scalar.activation(zs[:, TILE:2 * TILE], psZ[64:128, :], AF.Relu, bias=pc["czz"][64:128, :])
        nc.scalar.activation(zs[:, 2 * TILE:3 * TILE], psOP[:], AF.Relu)

        LHf = pc["LH"][:].bitcast(F32)
        psHa = ppool.tile([64, TILE], F32, tag="big")
        psHb = ppool.tile([64, TILE], F32, tag="med")
        psHc = pps.tile([8, TILE], F32, tag="sm")
        nc.tensor.matmul(psHa[0:8, :], LHf[:, 0:8], zs[:, 0:TILE], start=True, stop=True)
        nc.tensor.matmul(psHa[32:40, :], LHf[:, 8:16], zs[:, 0:TILE], start=True, stop=True)
        nc.tensor.matmul(psHb[0:8, :], LHf[:, 0:8], zs[:, TILE:2 * TILE], start=True, stop=True)
        nc.tensor.matmul(psHb[32:40, :], LHf[:, 8:16], zs[:, TILE:2 * TILE], start=True, stop=True)
        nc.tensor.matmul(psHc[:], LHf[:, 0:8], zs[:, 2 * TILE:3 * TILE], start=True, stop=True)
        l0 = wpool.tile([8, TILE], F32, tag="g_l0")
        l1 = wpool.tile([8, TILE], F32, tag="g_l1")
        l2 = wpool.tile([8, TILE], F32, tag="g_l2")
        nc.scalar.activation(l0[:], psHa[0:8, :], AF.Identity, bias=pc["c1p"])
        nc.scalar.activation(l1[:], psHb[0:8, :], AF.Identity, bias=pc["c1p"])
        nc.scalar.activation(l2[:], psHc[:], AF.Identity, bias=pc["c1p"])
        a0s = wpool.tile([8, TILE], F32, tag="g_a0s")
        nc.scalar.activation(a0s[:], psHa[32:40, :], AF.Copy)
        sd = wpool.tile([8, TILE], F32, tag="g_sd")
        nc.vector.tensor_tensor(sd[:], psHb[32:40, :], a0s[:], ALU.subtract)

        # per-row argmax -> slot/rule rows in transposed layout
        psT = pps.tile([128, 96], F32, tag="sm")
        for j in range(4):
            for t, lt in enumerate((l0, l1, l2)):
                nc.tensor.transpose(psT[:, 24 * j + 8 * t:24 * j + 8 * (t + 1)],
                                    lt[:, ts(j, 128)], id128[0:8, 0:8])
        vT = wpool.tile([128, 96], F32, tag="g_vT")
        nc.scalar.activation(vT[:], psT[:], AF.Copy)
        tn = wpool.tile([128, 2], F32, tag="g_tn")      # per-chunk [t, n] cols
        tr_t = wpool.tile([1, TILE], F32, tag="g_trt")  # slot row
        tr_n = wpool.tile([1, TILE], F32, tag="g_trn")  # rule row
        for j in range(4):
            mxj = wpool.tile([128, 8], F32, tag="g_mxj")
            mij = wpool.tile([128, 8], U32, tag="g_mij")
            nc.vector.max_with_indices(mxj[:], mij[:], vT[:, 24 * j:24 * (j + 1)])
            jf = wpool.tile([128, 1], F32, tag="g_jf")
            nc.vector.tensor_copy(jf[:], mij[:, 0:1])
            # t = 1{j>=8} + 1{j>=16};  n = j - 8t
            t1 = wpool.tile([128, 1], F32, tag="g_t1")
            nc.vector.tensor_scalar(t1[:], jf[:], 7.5, None, ALU.is_gt)
            t2 = wpool.tile([128, 1], F32, tag="g_t2")
            nc.vector.tensor_scalar(t2[:], jf[:], 15.5, None, ALU.is_gt)
            nc.vector.tensor_tensor(tn[:, 0:1], t1[:], t2[:], ALU.add)
            t8 = wpool.tile([128, 1], F32, tag="g_t8")
            nc.vector.tensor_scalar(t8[:], tn[:, 0:1], -8.0, None, ALU.mult)
            nc.vector.tensor_tensor(tn[:, 1:2], jf[:], t8[:], ALU.add)
            psTT = pps.tile([1, 128], F32, tag="sm")
            nc.tensor.transpose(psTT[:], tn[:, 0:1], id128[:])
            nc.scalar.activation(tr_t[:, ts(j, 128)], psTT[:], AF.Copy)
            psTN = pps.tile([1, 128], F32, tag="sm")
            nc.tensor.transpose(psTN[:], tn[:, 1:2], id128[:])
            nc.scalar.activation(tr_n[:, ts(j, 128)], psTN[:], AF.Copy)

        # rule one-hot (transposed) + slot row broadcast to 64 partitions
        psNB = pps.tile([8, TILE], F32, tag="sm")
        nc.tensor.matmul(psNB[:], onesr[:, 0:8], tr_n[:], start=True, stop=True)
        ohT8 = wpool.tile([8, TILE], F32, tag="g_ohT8")
        nc.vector.tensor_tensor(ohT8[:], iota8[:], psNB[:], ALU.is_equal)
        psTB = ppool.tile([64, TILE], F32, tag="med")
        nc.tensor.matmul(psTB[:], onesr[:, 0:64], tr_t[:], start=True, stop=True)
        tslot = wpool.tile([64, TILE], F32, tag="g_tslot")
        nc.scalar.activation(tslot[:], psTB[:], AF.Copy)

        # zp = sum_t zs_t * 1{slot==t}
        zp = wpool.tile([64, TILE], F32, tag="g_zp")
        acc = wpool.tile([64, TILE], F32, tag="g_acc")
        ohm = wpool.tile([64, TILE], F32, tag="g_ohm")
        for t in range(3):
            nc.vector.tensor_scalar(ohm[:], tslot[:], float(t), None, ALU.is_equal)
            dst = zp if t == 0 else acc
            nc.vector.tensor_tensor(dst[:], zs[:, t * TILE:(t + 1) * TILE], ohm[:], ALU.mult)
            if t > 0:
                nc.vector.tensor_tensor(zp[:], zp[:], acc[:], ALU.add)
        sdm = wpool.tile([8, TILE], F32, tag="g_sdm")
        nc.vector.tensor_tensor(sdm[:], sd[:], ohT8[:], ALU.mult)
        psDR = pps.tile([1, TILE], F32, tag="sm")
        nc.tensor.matmul(psDR[:], ones[0:8, :], sdm[:], start=True, stop=True)
        m1 = wpool.tile([1, TILE], F32, tag="g_m1")
        nc.vector.tensor_scalar(m1[:], psDR[:], 0.0, None, ALU.is_gt)
        psM1B = ppool.tile([64, TILE], F32, tag="med")
        nc.tensor.matmul(psM1B[:], onesr[:, 0:64], m1[:], start=True, stop=True)
        m1bi = wpool.tile([64, TILE], I32, tag="g_m1bi")
        nc.vector.tensor_copy(m1bi[:], psM1B[:])
        vc = wpool.tile([64, TILE], F32, tag="g_vc")
        nc.vector.select(vc[:], m1bi[:], zs[:, TILE:2 * TILE], zs[:, 0:TILE])

        # all experts, masked combine
        oacc = wpool.tile([1, TILE], F32, tag="g_oacc")
        me = wpool.tile([1, TILE], F32, tag="g_me")
        fm = wpool.tile([1, TILE], F32, tag="g_fm")
        for e in range(NR):
            psE = ppool.tile([128, TILE], F32, tag="big")
            nc.tensor.matmul(psE[:], pc["W1a"][:, ts(e, 128)], zp[:], start=True, stop=False)
            nc.tensor.matmul(psE[:], pc["W1b"][:, ts(e, 128)], vc[:], start=False, stop=True)
            he = wpool.tile([128, TILE], F32, tag="g_he")
            nc.scalar.activation(he[:], psE[:], AF.Relu, bias=pc["b1e"][:, e:e + 1])
            psO = ppool.tile([64, TILE], F32, tag="med")
            nc.tensor.matmul(psO[:], pc["W2f"][:, ts(e, 64)], he[:], start=True, stop=True)
            ho = wpool.tile([64, TILE], F32, tag="g_ho")
            nc.scalar.activation(ho[:], psO[:], AF.Relu, bias=pc["b2e"][:, e:e + 1])
            psF = pps.tile([1, TILE], F32, tag="sm")
            nc.tensor.matmul(psF[:], pc["LF"].bitcast(F32), ho[:], start=True, stop=True)
            fe = wpool.tile([1, TILE], F32, tag="g_fe")
            nc.scalar.activation(fe[:], psF[:], AF.Identity, bias=pc["deb2"][0:1, 0:1])
            nc.vector.tensor_scalar(me[:], tr_n[:], float(e), None, ALU.is_equal)
            nc.vector.tensor_tensor(fm[:], fe[:], me[:], ALU.mult)
            if e == 0:
                nc.vector.tensor_copy(oacc[:], fm[:])
            else:
                nc.vector.tensor_tensor(oacc[:], oacc[:], fm[:], ALU.add)
        nc.vector.tensor_copy(out_sb[:, ch], oacc[:])


def _shard_inputs(operand1, operand2, operator, PAR):
    o1 = np.asarray(operand1, np.float32)
    o2 = np.asarray(operand2, np.float32)
    opf = np.asarray(operator).astype(np.float32)
    in_maps = []
    for c in range(NCORES):
        sl = slice(c * RPC, (c + 1) * RPC)
        A = np.stack([o1[sl], np.ones(RPC, np.float32), o2[sl]])
        op3 = np.repeat(opf[sl][None, :], 3, axis=0)
        in_maps.append({"A": A, "op3": op3, "PAR": PAR})
    return in_maps


_NC_CACHE = {}


def kernel(operand1, operand2, operator, params):
    P = {k: np.asarray(v) for k, v in params.items()}
    PAR = _fold_params(P)
    in_maps = _shard_inputs(operand1, operand2, operator, PAR)
    if "nc" not in _NC_CACHE:
        _NC_CACHE["nc"] = build_nc()
    res = run_bass_kernel_spmd(_NC_CACHE["nc"], in_maps, core_ids=list(range(NCORES)))
    out = np.concatenate([res.results[c]["out"].reshape(-1) for c in range(NCORES)])
    return out.astype(np.float32)


if __name__ == "__main__":
    d = np.load("/tmp/inputs.npz")
    params = {k: d[k] for k in d.files if k not in ("operand1", "operand2", "operator")}
    out = kernel(d["operand1"], d["operand2"], d["operator"], params)
    exp = np.load("/tmp/expected.npy")
    rel = np.linalg.norm(out - exp) / np.linalg.norm(exp)
    print("Relative error:", rel)
